# revision 1
# baseline (speedup 1.0000x reference)
"""Trainium2 Bass kernel for nn_AttentionNet (pooling / ridge regime).

Model (per batch b of B=128, L=512, D=300, H=200, V=50000):
  word_emb = emb_table[words]                          [B,L,D]
  subj_emb = max over l with subj_pos[b,l]==0 of word_emb (else -1e12)
  obj_emb  = same with obj_pos
  hid  = tanh(word_emb @ w1[:D] + subj_emb @ w1[D:] + b1)
  attn = softmax(hid @ w2, axis=l)    (b2 dropped: softmax shift-invariant)
  subj_attn = sum_l attn * word_emb   (obj_attn identical -- source bug)
  out = relu(relu(cat([subj_attn, subj_attn, subj_emb, obj_emb]) @ mw1 + mb1) @ mw2 + mb2)

Sharding: pure data parallel, 16 batches per core on 8 cores; embedding
table and the small weights replicated.

Device plan per core (16 batches = 16 token-tiles of 512):
  - bulk gather via gpsimd.dma_gather (int16 indices).  The vocabulary
    exceeds int16 range, so each batch's tokens are sorted by word id
    (attention + pools are order-invariant within a batch) and split into
    the 256 smallest / 256 largest; the low halves of all batches are
    gathered from table[0:32768] and the high halves from
    table[V-32768:V], giving pure int16 indices with zero waste.
  - masked max-pools computed from the gathered embeddings: per-token
    additive masks (-2e12 for suppressed positions) via ACT/GPSIMD, a
    pairwise max tree on DVE/GPSIMD, PE transposes, a segmented
    reduce_max, then a final clamp to -1e12 which restores bit-exact
    semantics even for all-masked rows.
  - attention scores via fp32r matmuls on D-major PE-transposed
    embeddings, softmax on-chip, attention-weighted sum with the
    attention column as the stationary matmul operand.
  - 2-layer output MLP with the duplicated subj_attn block pre-folded
    into mw1 on the host (rows 0:300 += rows 300:600).
"""

import numpy as np

import concourse.bass as bass
import concourse.bacc as bacc
import concourse.mybir as mybir
import concourse.tile as tile
from concourse.masks import make_identity
from contextlib import ExitStack

F32 = mybir.dt.float32
F32R = mybir.dt.float32r
I16 = mybir.dt.int16

NEG_INF = 1e12      # reference constant
MASK_ADD = -2e12    # additive mask; clamped back to -NEG_INF at the end

# ---------------------------------------------------------------- config


class Cfg:
    def __init__(self, B=128, L=512, D=300, H=200, V=50000, NCORES=8,
                 PT=128, CW=128, HCW=100, use_f32r=True, gather_split=4):
        self.B, self.L, self.D, self.H, self.V = B, L, D, H, V
        self.NCORES = NCORES
        self.use_f32r = use_f32r
        self.BC = B // NCORES          # batches per core
        self.PT = PT                   # token subtile (partitions)
        self.NSUB = L // PT            # subtiles per batch (must be even)
        self.NS = self.BC * self.NSUB  # token subtiles per core
        self.T = self.BC * L           # tokens per core
        self.CW = CW                   # D-chunk width
        self.HCW = HCW                 # H-chunk width
        self.gather_split = gather_split
        assert L % PT == 0 and H % HCW == 0 and self.NSUB % 2 == 0
        # gather element size: row bytes padded to 256B multiples
        self.E = -(-D * 4 // 256) * 64
        # int16-addressable split of the vocabulary
        self.LO_MAX = min(V, 32768)    # low table = rows [0, LO_MAX)
        self.HB = max(V - 32768, 0)    # high table = rows [HB, V)
        # exact chunks of D (last may be narrow)
        self.dch = []
        s = 0
        while s < D:
            self.dch.append((s, min(CW, D - s)))
            s += CW
        self.hch = [(i * HCW, HCW) for i in range(H // HCW)]
        self.nd = len(self.dch)
        self.nh = len(self.hch)
        # transpose window start per chunk (narrow last chunk reads an
        # overlapping window ending at the padded width E; its rows sit at
        # a 32-aligned base so downstream APs stay legal)
        self.ov0 = [min(i * CW, self.E - CW) for i in range(self.nd)]
        self.r0 = [self.dch[i][0] - self.ov0[i] for i in range(self.nd)]
        for r, (d0, dn) in zip(self.r0, self.dch):
            assert r in (0, 32, 64, 96) and (r == 0 or dn <= max(32, 128 - r)), (r, dn)
        # pool-transpose source width padded to nd*CW (extra cols memset)
        self.DP = self.nd * CW

    def subtiles(self, b):
        """Global subtile ids of batch b: low half then high half."""
        h = self.NSUB // 2
        lo = [h * b + k for k in range(h)]
        hi = [self.NS // 2 + h * b + k for k in range(h)]
        return lo + hi


# ------------------------------------------------------------- device IR


def build_nc(cfg: Cfg):
    c = cfg
    FR = F32R if c.use_f32r else F32
    nc = bacc.Bacc(num_swdge_queues=4)

    NH16 = (c.T // 2) // 16
    idxlo_d = nc.declare_dram_parameter("idx_lo", [128, NH16], I16, isOutput=False)
    idxhi_d = nc.declare_dram_parameter("idx_hi", [128, NH16], I16, isOutput=False)
    table = nc.declare_dram_parameter("table", [c.V, c.E], FR, isOutput=False)
    madd_d = nc.declare_dram_parameter("madd", [c.PT, 2, c.BC, c.NSUB], F32, isOutput=False)
    w1a_d = nc.declare_dram_parameter("w1a", [c.D, c.H], F32, isOutput=False)
    w1b_d = nc.declare_dram_parameter("w1b", [c.D, c.H], F32, isOutput=False)
    b1_d = nc.declare_dram_parameter("b1", [c.H, 1], F32, isOutput=False)
    w2_d = nc.declare_dram_parameter("w2", [c.H, 1], F32, isOutput=False)
    mw1_d = nc.declare_dram_parameter("mw1e", [3 * c.D, c.H], F32, isOutput=False)
    mb1_d = nc.declare_dram_parameter("mb1", [c.H, 1], F32, isOutput=False)
    mw2_d = nc.declare_dram_parameter("mw2", [c.H, c.H], F32, isOutput=False)
    mb2_d = nc.declare_dram_parameter("mb2", [c.H, 1], F32, isOutput=False)
    out_d = nc.declare_dram_parameter("out", [c.nh, c.HCW, c.BC], F32, isOutput=True)

    with tile.TileContext(nc) as tc, ExitStack() as ctx:
        sb = ctx.enter_context(tc.tile_pool(name="sb", bufs=1))

        # ---- persistent SBUF tiles
        ixl_sb = sb.tile([128, NH16], I16)
        ixh_sb = sb.tile([128, NH16], I16)
        emb_tok = sb.tile([c.PT, c.NS, c.E], FR)
        madd_sb = sb.tile([c.PT, 2, c.BC, c.NSUB], F32)
        w1a_sb = sb.tile([c.CW, c.nd, c.H], F32)
        w1a_r = sb.tile([c.CW, c.nd, c.H], FR)
        w1b_sb = sb.tile([c.CW, c.nd, c.H], F32)
        w2_sb = sb.tile([c.HCW, c.nh], F32)
        w2_r = sb.tile([c.HCW, c.nh], FR)
        b1_sb = sb.tile([c.HCW, c.nh], F32)
        mw1_sb = sb.tile([c.CW, 3 * c.nd, c.H], F32)
        mb1_sb = sb.tile([c.HCW, c.nh], F32)
        mw2_sb = sb.tile([c.HCW, c.nh, c.H], F32)
        mb2_sb = sb.tile([c.HCW, c.nh], F32)
        ident = sb.tile([c.PT, c.PT], F32)
        identr = sb.tile([c.PT, c.PT], FR)
        pooled = sb.tile([c.CW, 2, c.BC, c.nd], F32)   # [dlow, mask, b, chunk]
        bias_sb = sb.tile([c.HCW, c.nh, c.BC], F32)
        scores = sb.tile([c.BC, c.L], F32)
        attn = sb.tile([c.BC, c.L], F32)
        attn_t = sb.tile([c.PT, c.NSUB, c.BC], FR)
        sattn = sb.tile([c.BC, c.D], F32)
        satd = sb.tile([c.CW, c.nd, c.BC], F32)
        smax = sb.tile([c.BC, 1], F32)
        nsmax = sb.tile([c.BC, 1], F32)
        ssum = sb.tile([c.BC, 1], F32)
        srec = sb.tile([c.BC, 1], F32)
        o1_sb = sb.tile([c.HCW, c.nh, c.BC], F32)
        out_sb = sb.tile([c.HCW, c.nh, c.BC], F32)

        # ---- load indices & weights
        nc.sync.dma_start(out=ixl_sb[:], in_=idxlo_d[:])
        nc.sync.dma_start(out=ixh_sb[:], in_=idxhi_d[:])
        nc.sync.dma_start(out=madd_sb[:], in_=madd_d[:])
        for ci, (d0, dn) in enumerate(c.dch):
            r0 = c.r0[ci]
            nc.sync.dma_start(out=w1a_sb[r0:r0 + dn, ci, :], in_=w1a_d[d0:d0 + dn, :])
            nc.vector.tensor_copy(out=w1a_r[r0:r0 + dn, ci, :],
                                  in_=w1a_sb[r0:r0 + dn, ci, :])
            nc.sync.dma_start(out=w1b_sb[0:dn, ci, :], in_=w1b_d[d0:d0 + dn, :])
            for blk in range(3):
                nc.sync.dma_start(out=mw1_sb[0:dn, blk * c.nd + ci, :],
                                  in_=mw1_d[blk * c.D + d0:blk * c.D + d0 + dn, :])
        for hi, (h0, hn) in enumerate(c.hch):
            nc.sync.dma_start(out=w2_sb[0:hn, hi:hi + 1], in_=w2_d[h0:h0 + hn, :])
            nc.vector.tensor_copy(out=w2_r[0:hn, hi:hi + 1], in_=w2_sb[0:hn, hi:hi + 1])
            nc.sync.dma_start(out=b1_sb[0:hn, hi:hi + 1], in_=b1_d[h0:h0 + hn, :])
            nc.sync.dma_start(out=mb1_sb[0:hn, hi:hi + 1], in_=mb1_d[h0:h0 + hn, :])
            nc.sync.dma_start(out=mb2_sb[0:hn, hi:hi + 1], in_=mb2_d[h0:h0 + hn, :])
            nc.sync.dma_start(out=mw2_sb[0:hn, hi, :], in_=mw2_d[h0:h0 + hn, :])
        make_identity(nc, ident[:])
        nc.vector.tensor_copy(out=identr[:], in_=ident[:])

        # ---- bulk gathers: low halves -> subtiles [0, NS/2), high halves after
        NHALF = c.T // 2
        nsp = c.gather_split
        npc = NHALF // nsp
        assert npc % 128 == 0, (NHALF, nsp)
        for k in range(nsp):
            i0, s0 = k * (npc // 16), k * (npc // 128)
            nc.gpsimd.dma_gather(
                out_ap=emb_tok[:, s0:s0 + npc // 128, :], in_ap=table[0:c.LO_MAX, :],
                idxs_ap=ixl_sb[:, i0:i0 + npc // 16], num_idxs=npc, num_idxs_reg=npc,
                elem_size=c.E, single_packet=False, queue_num=(2 * k) % 4 if c.use_f32r else 0)
            nc.gpsimd.dma_gather(
                out_ap=emb_tok[:, c.NS // 2 + s0:c.NS // 2 + s0 + npc // 128, :],
                in_ap=table[c.HB:c.V, :],
                idxs_ap=ixh_sb[:, i0:i0 + npc // 16], num_idxs=npc, num_idxs_reg=npc,
                elem_size=c.E, single_packet=False, queue_num=(2 * k + 1) % 4)

        # ---- grouped main loop: pools -> group bias -> dense hid/scores.
        # Groups of GB batches keep the PE stream dense (HAM stays warm) and
        # let the ACT/DVE pool work of group g+1 overlap the PE work of g.
        GB = min(4, c.BC)
        NG = c.BC // GB

        def pool_rhs(m, ci, bsl=slice(None)):
            dn = c.dch[ci][1]
            return pooled[0:dn, m, bsl, ci]

        with tc.tile_pool(name="mkpool", bufs=3) as mkpool, \
             tc.tile_pool(name="mxpool", bufs=3) as mxpool, \
             tc.tile_pool(name="ppool", bufs=1, space="PSUM") as ppool, \
             tc.tile_pool(name="bpool", bufs=1, space="PSUM") as bpool, \
             tc.tile_pool(name="tpool", bufs=1, space="PSUM") as tpool, \
             tc.tile_pool(name="hpool", bufs=2, space="PSUM") as hpool, \
             tc.tile_pool(name="spool", bufs=1, space="PSUM") as spool, \
             tc.tile_pool(name="epool", bufs=2) as epool, \
             tc.tile_pool(name="hspool", bufs=2) as hspool, \
             tc.tile_pool(name="srpool", bufs=3) as srpool:
            for g in range(NG):
                gsl = slice(g * GB, (g + 1) * GB)
                # -- pools for the group
                for b in range(g * GB, (g + 1) * GB):
                    subs = c.subtiles(b)
                    for m in range(2):
                        masked = mkpool.tile([c.PT, c.NSUB, c.D], F32, tag="masked")
                        for si, s in enumerate(subs):
                            if m == 0:
                                nc.scalar.activation(
                                    out=masked[:, si, :],
                                    in_=emb_tok[:, s, 0:c.D].bitcast(F32),
                                    func=mybir.ActivationFunctionType.Identity,
                                    bias=madd_sb[:, m, b, si:si + 1], scale=1.0)
                            else:
                                nc.vector.tensor_scalar(
                                    out=masked[:, si, :],
                                    in0=emb_tok[:, s, 0:c.D].bitcast(F32),
                                    scalar1=madd_sb[:, m, b, si:si + 1],
                                    scalar2=None, op0=mybir.AluOpType.add)
                        h = c.NSUB // 2
                        maxed = mxpool.tile([c.PT, c.DP], F32, tag="maxed")
                        t1 = mkpool.tile([c.PT, h, c.D], F32, tag="t1")
                        nc.vector.tensor_tensor(out=t1[:], in0=masked[:, 0:h, :],
                                                in1=masked[:, h:c.NSUB, :],
                                                op=mybir.AluOpType.max)
                        for q in range(h.bit_length() - 1):
                            hh = h >> (q + 1)
                            nc.vector.tensor_tensor(
                                out=t1[:, 0:hh, :], in0=t1[:, 0:hh, :],
                                in1=t1[:, hh:2 * hh, :], op=mybir.AluOpType.max)
                        nc.gpsimd.memset(maxed[:, c.D:c.DP], MASK_ADD)
                        nc.vector.tensor_copy(out=maxed[:, 0:c.D], in_=t1[:, 0, :])
                        pp = ppool.tile([c.CW, c.nd, c.PT], F32, tag="pp")
                        for ci in range(c.nd):
                            nc.tensor.transpose(
                                out=pp[:, ci, :],
                                in_=maxed[:, ci * c.CW:(ci + 1) * c.CW],
                                identity=ident[:])
                        nc.vector.tensor_reduce(
                            out=pooled[:, m, b, :], in_=pp[:],
                            axis=mybir.AxisListType.X, op=mybir.AluOpType.max)
                # -- clamp restores exact -1e12 for all-masked rows
                nc.vector.tensor_scalar_max(
                    out=pooled[:, :, gsl, :], in0=pooled[:, :, gsl, :],
                    scalar1=-NEG_INF)
                # -- tanh bias for the group: w1b^T subj_emb + b1
                for hi, (h0, hn) in enumerate(c.hch):
                    pb = bpool.tile([c.HCW, GB], F32, tag="pb")
                    for ci, (d0, dn) in enumerate(c.dch):
                        nc.tensor.matmul(
                            out=pb[0:hn, :],
                            lhsT=w1b_sb[0:dn, ci, h0:h0 + hn],
                            rhs=pool_rhs(0, ci, gsl),
                            start=(ci == 0), stop=(ci == c.nd - 1))
                    nc.scalar.activation(
                        out=bias_sb[0:hn, hi, gsl], in_=pb[0:hn, :],
                        func=mybir.ActivationFunctionType.Identity,
                        bias=b1_sb[0:hn, hi:hi + 1], scale=1.0)
                # -- D-major transposes + copies for the group
                embds = []
                for b in range(g * GB, (g + 1) * GB):
                    subs = c.subtiles(b)
                    pt = tpool.tile([c.CW, c.nd, c.L], FR, tag="pt")
                    for ci in range(c.nd):
                        o0 = c.ov0[ci]
                        for si, s in enumerate(subs):
                            nc.tensor.transpose(
                                out=pt[:, ci, si * c.PT:(si + 1) * c.PT],
                                in_=emb_tok[:, s, o0:o0 + c.CW],
                                identity=identr[:])
                    emb_d = epool.tile([c.CW, c.nd, c.L], FR, tag="embd")
                    nc.scalar.copy(out=emb_d[:, 0, :], in_=pt[:, 0, :])
                    nc.vector.tensor_copy(out=emb_d[:, 1:c.nd, :], in_=pt[:, 1:c.nd, :])
                    embds.append(emb_d)
                # -- dense hid + scores matmul stream for the group
                for bi, b in enumerate(range(g * GB, (g + 1) * GB)):
                    emb_d = embds[bi]
                    hid = hspool.tile([c.HCW, c.nh, c.L], FR, tag="hid")
                    for hi, (h0, hn) in enumerate(c.hch):
                        ph = hpool.tile([c.HCW, c.L], F32, tag="ph")
                        for ci, (d0, dn) in enumerate(c.dch):
                            r0 = c.r0[ci]
                            nc.tensor.matmul(
                                out=ph[0:hn, :],
                                lhsT=w1a_r[r0:r0 + dn, ci, h0:h0 + hn],
                                rhs=emb_d[r0:r0 + dn, ci, :],
                                start=(ci == 0), stop=(ci == c.nd - 1))
                        nc.scalar.activation(
                            out=hid[0:hn, hi, :], in_=ph[0:hn, :],
                            func=mybir.ActivationFunctionType.Tanh,
                            bias=bias_sb[0:hn, hi, b:b + 1], scale=1.0)
                    ps = spool.tile([1, c.L], F32, tag="ps")
                    for hi, (h0, hn) in enumerate(c.hch):
                        nc.tensor.matmul(
                            out=ps[:], lhsT=w2_r[0:hn, hi:hi + 1],
                            rhs=hid[0:hn, hi, :],
                            start=(hi == 0), stop=(hi == c.nh - 1))
                    srow = srpool.tile([1, c.L], F32, tag="srow")
                    nc.vector.tensor_copy(out=srow[:], in_=ps[:])
                    nc.sync.dma_start(out=scores[b:b + 1, :], in_=srow[:])

        # ---- softmax over L for all batches
        nc.vector.tensor_reduce(out=smax[:], in_=scores[:],
                                axis=mybir.AxisListType.X, op=mybir.AluOpType.max)
        nc.vector.tensor_scalar_mul(out=nsmax[:], in0=smax[:], scalar1=-1.0)
        nc.scalar.activation(out=attn[:], in_=scores[:],
                             func=mybir.ActivationFunctionType.Exp,
                             bias=nsmax[:, 0:1], scale=1.0)
        nc.vector.tensor_reduce(out=ssum[:], in_=attn[:],
                                axis=mybir.AxisListType.X, op=mybir.AluOpType.add)
        nc.vector.reciprocal(out=srec[:], in_=ssum[:])
        nc.vector.tensor_scalar_mul(out=attn[:], in0=attn[:], scalar1=srec[:, 0:1])

        # ---- transpose attn to token-major columns [PT, si, b]
        # column layout: attn[b, si*PT:...] -> attn_t[:, si, b]
        with tc.tile_pool(name="apool", bufs=2, space="PSUM") as apool:
            for si in range(c.NSUB):
                pa = apool.tile([c.PT, c.BC], F32, tag="pa")
                nc.tensor.transpose(out=pa[:],
                                    in_=attn[:, si * c.PT:(si + 1) * c.PT],
                                    identity=ident[0:c.BC, 0:c.BC])
                nc.vector.tensor_copy(out=attn_t[:, si, :], in_=pa[:])

        # ---- attention-weighted sum  -> sattn [b, D]
        with tc.tile_pool(name="wpool", bufs=4, space="PSUM") as wpool, \
             tc.tile_pool(name="wrpool", bufs=3) as wrpool:
            for b in range(c.BC):
                subs = c.subtiles(b)
                pw = wpool.tile([1, c.D], F32, tag="pw")
                for si, s in enumerate(subs):
                    nc.tensor.matmul(
                        out=pw[:],
                        lhsT=attn_t[:, si, b:b + 1],
                        rhs=emb_tok[:, s, 0:c.D],
                        start=(si == 0), stop=(si == c.NSUB - 1))
                wrow = wrpool.tile([1, c.D], F32, tag="wrow")
                nc.scalar.copy(out=wrow[:], in_=pw[:])
                nc.sync.dma_start(out=sattn[b:b + 1, :], in_=wrow[:])

        # ---- transpose sattn to D-major chunks [dlow, chunk, b]
        with tc.tile_pool(name="stpool", bufs=2, space="PSUM") as stpool:
            for ci, (d0, dn) in enumerate(c.dch):
                pst = stpool.tile([c.CW, c.BC], F32, tag="pst")
                nc.tensor.transpose(out=pst[0:dn, :], in_=sattn[:, d0:d0 + dn],
                                    identity=ident[0:c.BC, 0:c.BC])
                nc.vector.tensor_copy(out=satd[0:dn, ci, :], in_=pst[0:dn, :])

        # ---- output MLP (fp32; N=BC is small)
        with tc.tile_pool(name="mpool", bufs=2, space="PSUM") as mpool, \
             tc.tile_pool(name="m2pool", bufs=2, space="PSUM") as m2pool:
            nk = 3 * c.nd
            for hi, (h0, hn) in enumerate(c.hch):
                pm = mpool.tile([c.HCW, c.BC], F32, tag="pm")
                for blk in range(3):
                    for ci, (d0, dn) in enumerate(c.dch):
                        k = blk * c.nd + ci
                        if blk == 0:
                            rhs = satd[0:dn, ci, :]
                        else:
                            rhs = pool_rhs(blk - 1, ci)
                        nc.tensor.matmul(
                            out=pm[0:hn, :],
                            lhsT=mw1_sb[0:dn, k, h0:h0 + hn],
                            rhs=rhs, start=(k == 0), stop=(k == nk - 1))
                nc.scalar.activation(
                    out=o1_sb[0:hn, hi, :], in_=pm[0:hn, :],
                    func=mybir.ActivationFunctionType.Relu,
                    bias=mb1_sb[0:hn, hi:hi + 1], scale=1.0)
            for hi, (h0, hn) in enumerate(c.hch):
                pm2 = m2pool.tile([c.HCW, c.BC], F32, tag="pm2")
                for ki, (k0, kn) in enumerate(c.hch):
                    nc.tensor.matmul(
                        out=pm2[0:hn, :],
                        lhsT=mw2_sb[0:kn, ki, h0:h0 + hn],
                        rhs=o1_sb[0:kn, ki, :],
                        start=(ki == 0), stop=(ki == c.nh - 1))
                nc.scalar.activation(
                    out=out_sb[0:hn, hi, :], in_=pm2[0:hn, :],
                    func=mybir.ActivationFunctionType.Relu,
                    bias=mb2_sb[0:hn, hi:hi + 1], scale=1.0)
            for hi in range(c.nh):
                nc.sync.dma_start(out=out_d[hi], in_=out_sb[:, hi, :])

    nc.finalize()
    return nc


# ------------------------------------------------------------------ host


def wrap16(idx, n):
    """int16 index list -> [128, n/16] wrapped + replicated per Q7 core."""
    return np.ascontiguousarray(
        np.tile(idx.astype(np.int16).reshape(n // 16, 16).T, (8, 1)))


def host_prepare(cfg: Cfg, words, subj_pos, obj_pos, emb_table,
                 w1, b1, w2, b2, mw1, mb1, mw2, mb2):
    c = cfg
    words = np.asarray(words).astype(np.int64)
    subj_pos = np.asarray(subj_pos)
    obj_pos = np.asarray(obj_pos)
    f32 = lambda x: np.ascontiguousarray(np.asarray(x, dtype=np.float32))

    table = np.zeros((c.V, c.E), np.float32)
    table[:, :c.D] = np.asarray(emb_table, dtype=np.float32)

    w1 = f32(w1)
    w1a, w1b = w1[:c.D], w1[c.D:2 * c.D]
    mw1 = f32(mw1)
    mw1e = np.concatenate([mw1[0:c.D] + mw1[c.D:2 * c.D],
                           mw1[2 * c.D:3 * c.D], mw1[3 * c.D:4 * c.D]], axis=0)
    shared = {
        "table": table,
        "w1a": f32(w1a), "w1b": f32(w1b),
        "b1": f32(b1).reshape(c.H, 1),
        "w2": f32(w2).reshape(c.H, 1),
        "mw1e": f32(mw1e),
        "mb1": f32(mb1).reshape(c.H, 1),
        "mw2": f32(mw2),
        "mb2": f32(mb2).reshape(c.H, 1),
    }
    HALF = c.L // 2
    in_maps = []
    for core in range(c.NCORES):
        b0 = core * c.BC
        lo_list, hi_list = [], []
        madd = np.zeros((c.PT, 2, c.BC, c.NSUB), np.float32)
        for b in range(c.BC):
            w = words[b0 + b]
            order = np.argsort(w, kind="stable")
            ws = w[order]
            if ws[HALF - 1] >= c.LO_MAX or ws[HALF] < c.HB:
                raise RuntimeError(
                    f"batch {b0 + b}: vocab split infeasible "
                    f"(lo_max={ws[HALF - 1]}, hi_min={ws[HALF]})")
            lo_list.append(ws[:HALF])
            hi_list.append(ws[HALF:] - c.HB)
            # mask addends follow the same permutation; token rank r sits at
            # subtile si=r//PT (low half) / NSUB/2 + (r-HALF)//PT, partition r%PT
            for m, pos in ((0, subj_pos), (1, obj_pos)):
                pm = (np.asarray(pos[b0 + b])[order] != 0)
                av = np.where(pm, np.float32(MASK_ADD), np.float32(0.0))
                madd[:, m, b, :] = av.reshape(c.NSUB, c.PT).T
        idx_lo = np.concatenate(lo_list)
        idx_hi = np.concatenate(hi_list)
        in_maps.append({
            "idx_lo": wrap16(idx_lo, c.T // 2),
            "idx_hi": wrap16(idx_hi, c.T // 2),
            "madd": np.ascontiguousarray(madd),
            **shared})
    return in_maps


def assemble_output(cfg: Cfg, results):
    c = cfg
    outs = []
    for core in range(c.NCORES):
        o = results[core]["out"]                      # [nh, HCW, BC]
        outs.append(o.reshape(c.H, c.BC).T)           # [BC, H]
    return np.ascontiguousarray(np.concatenate(outs, axis=0))


_CACHE = {}


def run(inputs, trace=False, **kw):
    from concourse.bass_utils import run_bass_kernel_spmd

    cfg = Cfg()
    in_maps = host_prepare(cfg, **{k: inputs[k] for k in (
        "words", "subj_pos", "obj_pos", "emb_table", "w1", "b1", "w2", "b2",
        "mw1", "mb1", "mw2", "mb2")})
    if "nc" not in _CACHE:
        _CACHE["nc"] = build_nc(cfg)
    nc = _CACHE["nc"]
    res = run_bass_kernel_spmd(nc, in_maps, core_ids=list(range(cfg.NCORES)),
                               trace=trace, **kw)
    return assemble_output(cfg, res.results), res


def kernel(**inputs) -> np.ndarray:
    return run(inputs)[0]



# revision 6
# speedup vs baseline: 1.0935x; 1.0935x over previous
"""Trainium2 Bass kernel for nn_AttentionNet (pooling / ridge regime).

Model (per batch b of B=128, L=512, D=300, H=200, V=50000):
  word_emb = emb_table[words]                          [B,L,D]
  subj_emb = max over l with subj_pos[b,l]==0 of word_emb (else -1e12)
  obj_emb  = same with obj_pos
  hid  = tanh(word_emb @ w1[:D] + subj_emb @ w1[D:] + b1)
  attn = softmax(hid @ w2, axis=l)    (b2 dropped: softmax shift-invariant)
  subj_attn = sum_l attn * word_emb   (obj_attn identical -- source bug)
  out = relu(relu(cat([subj_attn, subj_attn, subj_emb, obj_emb]) @ mw1 + mb1) @ mw2 + mb2)

Sharding: pure data parallel, 16 batches per core on 8 cores; embedding
table (bf16) and the small weights replicated.

Device plan per core (16 batches = 16 token-tiles of 512):
  - everything on-chip is bf16 (weights, embeddings) with fp32 PSUM /
    softmax / biases.  The table rows are padded to 384 bf16 columns
    (768 B, the 256B gather granularity) so D splits into exactly three
    128-row chunks with zero pad columns -- pad embedding values are 0.0
    and multiply zero weight rows everywhere downstream.
  - bulk gather via gpsimd.dma_gather (int16 indices, vocab sorted/split
    per batch into low/high 32768-row windows as before).  The gathers
    are issued first so SWDGE descriptor generation overlaps the weight
    loads, and compute chases gather splits group by group.
  - all weights ship in one packed [128, 3530] bf16 blob + one [128, 6]
    f32 bias blob (one DMA each instead of ~30 serialized descriptors).
  - per group of 4 batches: PE-transpose embeddings to D-major, dense
    hid/score matmuls, masked max-pools via broadcast mask-add + a short
    max tree, per-group softmax + attention-weighted sum so the
    attention tail pipelines with the next group's matmul stream.
"""

import numpy as np
import ml_dtypes

import concourse.bass as bass
import concourse.bacc as bacc
import concourse.mybir as mybir
import concourse.tile as tile
from concourse.masks import make_identity
from contextlib import ExitStack

F32 = mybir.dt.float32
BF16 = mybir.dt.bfloat16
I16 = mybir.dt.int16

NEG_INF = 1e12      # reference constant
MASK_ADD = -2e12    # additive mask; clamped back to -NEG_INF at the end
BFNP = ml_dtypes.bfloat16

AX = mybir.AxisListType.X
MAX = mybir.AluOpType.max
ADD = mybir.AluOpType.add
AF = mybir.ActivationFunctionType


class Cfg:
    def __init__(self):
        self.B, self.L, self.D, self.H, self.V = 128, 512, 300, 200, 50000
        self.NCORES = 8
        self.BC = 16                  # batches per core
        self.PT = 128                 # tokens per subtile
        self.NSUB = 4                 # subtiles per batch
        self.NS = 64                  # subtiles per core
        self.DP = 384                 # padded D (3 chunks of 128)
        self.E = 384                  # gather row elems (bf16) = 768 B
        self.HCW = 100                # H chunk width
        self.nh = 2
        self.nd = 3
        self.GB = 4                   # batches per group
        self.NG = 4
        self.HALF = 256               # tokens per vocab half
        self.LO_MAX = 32768
        self.HB = self.V - 32768      # high window base
        self.nsp = 4                  # gather splits per half
        # wb16 blob column offsets
        self.W1A = 0
        self.W1B = 600
        self.MW1 = 1200
        self.MW2 = 3000
        self.W2 = 3400
        self.MADD = 3402
        self.NC16 = 3402 + 128

    def subtiles(self, b):
        return [2 * b, 2 * b + 1, 32 + 2 * b, 32 + 2 * b + 1]


def build_nc(cfg: Cfg):
    c = cfg
    nc = bacc.Bacc(num_swdge_queues=4)

    idx_d = nc.declare_dram_parameter("idx", [128, 512], I16, isOutput=False)
    wb16_d = nc.declare_dram_parameter("wb16", [128, c.NC16], BF16, isOutput=False)
    wb32_d = nc.declare_dram_parameter("wb32", [128, 6], F32, isOutput=False)
    table = nc.declare_dram_parameter("table", [c.V, c.E], BF16, isOutput=False)
    out_d = nc.declare_dram_parameter("out", [c.nh, c.HCW, c.BC], F32, isOutput=True)

    with tile.TileContext(nc) as tc, ExitStack() as ctx:
        sb = ctx.enter_context(tc.tile_pool(name="sb", bufs=1))

        ix_sb = sb.tile([128, 512], I16)
        wb16_sb = sb.tile([128, c.NC16], BF16)
        wb32_sb = sb.tile([128, 6], F32)
        identb = sb.tile([128, 128], BF16)
        emb_tok = sb.tile([128, c.NS, c.E], BF16)
        pooledf = sb.tile([128, 2, c.nd, c.BC], BF16)
        pooled = sb.tile([128, 2, c.nd, c.BC], BF16)
        bias_sb = sb.tile([c.HCW, c.nh, c.BC], F32)
        attn_t = sb.tile([128, c.NSUB, c.BC], BF16)
        sattn_sb = sb.tile([c.BC, c.E], BF16)
        satd = sb.tile([128, c.nd, c.BC], BF16)
        o1_sb = sb.tile([c.HCW, c.nh, c.BC], BF16)
        out_sb = sb.tile([c.HCW, c.nh, c.BC], F32)

        # identity first (cheap, must not queue behind gather desc-gen)
        make_identity(nc, identb[:])

        # index DMA first: it gates gather descriptor generation
        nc.sync.dma_start(out=ix_sb[:], in_=idx_d[:])
        nc.sync.dma_start(out=wb16_sb[:], in_=wb16_d[:])
        nc.sync.dma_start(out=wb32_sb[:], in_=wb32_d[:])

        # gathers: lo/hi pair per split k; pair k feeds batch group k
        npc = c.nsp * 256 // c.nsp * (c.HALF * c.BC // c.nsp) // (c.HALF * c.BC // c.nsp)  # noqa
        npc = c.HALF * c.BC // c.nsp          # 1024 rows per gather
        nsubt = npc // 128                    # 8 subtiles per gather
        for k in range(c.nsp):
            i0 = k * (npc // 16)
            nc.gpsimd.dma_gather(
                out_ap=emb_tok[:, k * nsubt:(k + 1) * nsubt, :],
                in_ap=table[0:c.LO_MAX, :],
                idxs_ap=ix_sb[:, i0:i0 + npc // 16],
                num_idxs=npc, num_idxs_reg=npc, elem_size=c.E,
                single_packet=False, queue_num=(2 * k) % 4)
            nc.gpsimd.dma_gather(
                out_ap=emb_tok[:, 32 + k * nsubt:32 + (k + 1) * nsubt, :],
                in_ap=table[c.HB:c.V, :],
                idxs_ap=ix_sb[:, 256 + i0:256 + i0 + npc // 16],
                num_idxs=npc, num_idxs_reg=npc, elem_size=c.E,
                single_packet=False, queue_num=(2 * k + 1) % 4)

        def w16(col, n):
            return wb16_sb[:, col:col + n]

        with tc.tile_pool(name="edpool", bufs=8) as edpool, \
             tc.tile_pool(name="mkpool", bufs=3) as mkpool, \
             tc.tile_pool(name="hidpool", bufs=2) as hidpool, \
             tc.tile_pool(name="sstpool", bufs=2) as sstpool, \
             tc.tile_pool(name="saspool", bufs=2) as saspool, \
             tc.tile_pool(name="scgpool", bufs=2) as scgpool, \
             tc.tile_pool(name="smpool", bufs=2) as smpool, \
             tc.tile_pool(name="tepool", bufs=2, space="PSUM") as tepool, \
             tc.tile_pool(name="tppool", bufs=1, space="PSUM") as tppool, \
             tc.tile_pool(name="hpool", bufs=2, space="PSUM") as hpool, \
             tc.tile_pool(name="bpool", bufs=1, space="PSUM") as bpool, \
             tc.tile_pool(name="spool", bufs=1, space="PSUM") as spool:
            for g in range(c.NG):
                g0 = g * c.GB
                gsl = slice(g0, g0 + c.GB)
                eds = []
                for b in range(g0, g0 + c.GB):
                    subs = c.subtiles(b)
                    ed = edpool.tile([128, c.nd, c.L], BF16, tag="ed")
                    for ci in range(c.nd):
                        te = tepool.tile([128, c.NSUB, 128], BF16, tag="te")
                        for si, s in enumerate(subs):
                            nc.tensor.transpose(
                                out=te[:, si, :],
                                in_=emb_tok[:, s, ci * 128:(ci + 1) * 128],
                                identity=identb[:])
                        if ci == 0:
                            nc.scalar.copy(out=ed[:, ci, :], in_=te[:])
                        else:
                            nc.any.tensor_copy(out=ed[:, ci, :], in_=te[:])
                    eds.append(ed)
                    # masked max pools (token-major): broadcast mask add +
                    # short max tree + PE transpose + cross-partition reduce
                    for m in range(2):
                        mcol = c.MADD + (m * c.BC + b) * 4
                        mk = mkpool.tile([128, 2, 2, c.DP], BF16, tag="mk")
                        nc.any.tensor_tensor(
                            out=mk[:, 0, :, :],
                            in0=emb_tok[:, 2 * b:2 * b + 2, :],
                            in1=wb16_sb[:, mcol:mcol + 2].to_broadcast(
                                [128, 2, c.DP]),
                            op=ADD)
                        nc.any.tensor_tensor(
                            out=mk[:, 1, :, :],
                            in0=emb_tok[:, 32 + 2 * b:32 + 2 * b + 2, :],
                            in1=wb16_sb[:, mcol + 2:mcol + 4].to_broadcast(
                                [128, 2, c.DP]),
                            op=ADD)
                        nc.any.tensor_tensor(
                            out=mk[:, 0, :, :], in0=mk[:, 0, :, :],
                            in1=mk[:, 1, :, :], op=MAX)
                        nc.any.tensor_tensor(
                            out=mk[:, 0, 0, :], in0=mk[:, 0, 0, :],
                            in1=mk[:, 0, 1, :], op=MAX)
                        tp = tppool.tile([128, c.nd, 128], BF16, tag="tp")
                        for ci in range(c.nd):
                            nc.tensor.transpose(
                                out=tp[:, ci, :],
                                in_=mk[:, 0, 0, ci * 128:(ci + 1) * 128],
                                identity=identb[:])
                        nc.vector.tensor_reduce(
                            out=pooledf[:, m, :, b], in_=tp[:],
                            axis=AX, op=MAX)
                # clamp restores exact -1e12 for all-masked rows + bf16 cast
                nc.any.tensor_scalar_max(
                    out=pooled[:, :, :, gsl], in0=pooledf[:, :, :, gsl],
                    scalar1=-NEG_INF)
                # tanh bias for the group: w1b^T subj_emb + b1
                for hi in range(c.nh):
                    pb = bpool.tile([c.HCW, c.GB], F32, tag="pb")
                    for ci in range(c.nd):
                        nc.tensor.matmul(
                            out=pb[:],
                            lhsT=w16(c.W1B + ci * 200 + hi * 100, 100),
                            rhs=pooled[:, 0, ci, gsl],
                            start=(ci == 0), stop=(ci == c.nd - 1))
                    nc.scalar.activation(
                        out=bias_sb[:, hi, gsl], in_=pb[:],
                        func=AF.Identity, bias=wb32_sb[0:c.HCW, hi:hi + 1],
                        scale=1.0)
                # dense hid + scores per batch
                sst = sstpool.tile([1, c.GB, c.L], F32, tag="sst")
                for bi, b in enumerate(range(g0, g0 + c.GB)):
                    ed = eds[bi]
                    hid = hidpool.tile([c.HCW, c.nh, c.L], BF16, tag="hid")
                    for hi in range(c.nh):
                        ph = hpool.tile([c.HCW, c.L], F32, tag="ph")
                        for ci in range(c.nd):
                            nc.tensor.matmul(
                                out=ph[:],
                                lhsT=w16(c.W1A + ci * 200 + hi * 100, 100),
                                rhs=ed[:, ci, :],
                                start=(ci == 0), stop=(ci == c.nd - 1))
                        nc.scalar.activation(
                            out=hid[:, hi, :], in_=ph[:],
                            func=AF.Tanh, bias=bias_sb[:, hi, b:b + 1],
                            scale=1.0)
                    ps = spool.tile([1, c.L], F32, tag="ps")
                    for hi in range(c.nh):
                        nc.tensor.matmul(
                            out=ps[:],
                            lhsT=wb16_sb[0:c.HCW, c.W2 + hi:c.W2 + hi + 1],
                            rhs=hid[:, hi, :],
                            start=(hi == 0), stop=(hi == c.nh - 1))
                    nc.any.tensor_copy(out=sst[0:1, bi, :], in_=ps[:])
                # batch-major softmax for the group
                scg = scgpool.tile([c.GB, c.L], F32, tag="sc")
                nc.sync.dma_start(out=scg[:], in_=sst[:])
                sm = smpool.tile([c.GB, 4], F32, tag="sm")
                nc.vector.tensor_reduce(out=sm[:, 0:1], in_=scg[:], axis=AX, op=MAX)
                nc.any.tensor_scalar_mul(out=sm[:, 1:2], in0=sm[:, 0:1], scalar1=-1.0)
                ax = scgpool.tile([c.GB, c.L], F32, tag="ax")
                nc.scalar.activation(out=ax[:], in_=scg[:], func=AF.Exp,
                                     bias=sm[:, 1:2], scale=1.0)
                nc.vector.tensor_reduce(out=sm[:, 2:3], in_=ax[:], axis=AX, op=ADD)
                nc.vector.reciprocal(out=sm[:, 3:4], in_=sm[:, 2:3])
                atb = scgpool.tile([c.GB, c.L], BF16, tag="atb")
                nc.any.tensor_scalar_mul(out=atb[:], in0=ax[:], scalar1=sm[:, 3:4])
                for si in range(c.NSUB):
                    tp = tppool.tile([128, c.nd, 128], BF16, tag="tp")
                    nc.tensor.transpose(
                        out=tp[:, 0, 0:c.GB],
                        in_=atb[:, si * 128:(si + 1) * 128],
                        identity=identb[0:c.GB, 0:c.GB])
                    nc.any.tensor_copy(out=attn_t[:, si, gsl], in_=tp[:, 0, 0:c.GB])
                # attention-weighted sum
                sas = saspool.tile([1, c.GB, c.E], BF16, tag="sas")
                for bi, b in enumerate(range(g0, g0 + c.GB)):
                    pw = spool.tile([1, c.L], F32, tag="ps")
                    for si, s in enumerate(c.subtiles(b)):
                        nc.tensor.matmul(
                            out=pw[:, 0:c.E],
                            lhsT=attn_t[:, si, b:b + 1],
                            rhs=emb_tok[:, s, :],
                            start=(si == 0), stop=(si == c.NSUB - 1))
                    nc.any.tensor_copy(out=sas[0:1, bi, :], in_=pw[:, 0:c.E])
                nc.sync.dma_start(out=sattn_sb[gsl, :], in_=sas[:])

        # ---- tail: satd transposes + output MLP
        with tc.tile_pool(name="stpool", bufs=1, space="PSUM") as stpool, \
             tc.tile_pool(name="mpool", bufs=2, space="PSUM") as mpool:
            for ci in range(c.nd):
                pst = stpool.tile([128, c.BC], BF16, tag="pst")
                nc.tensor.transpose(
                    out=pst[:],
                    in_=sattn_sb[:, ci * 128:(ci + 1) * 128],
                    identity=identb[0:c.BC, 0:c.BC])
                nc.any.tensor_copy(out=satd[:, ci, :], in_=pst[:])
            for hi in range(c.nh):
                pm = mpool.tile([c.HCW, c.BC], F32, tag="pm")
                k = 0
                for blk in range(3):
                    for ci in range(c.nd):
                        rhs = satd[:, ci, :] if blk == 0 else pooled[:, blk - 1, ci, :]
                        nc.tensor.matmul(
                            out=pm[:],
                            lhsT=w16(c.MW1 + (blk * 3 + ci) * 200 + hi * 100, 100),
                            rhs=rhs, start=(k == 0), stop=(k == 8))
                        k += 1
                nc.scalar.activation(
                    out=o1_sb[:, hi, :], in_=pm[:], func=AF.Relu,
                    bias=wb32_sb[0:c.HCW, 2 + hi:3 + hi], scale=1.0)
            for hi in range(c.nh):
                pm2 = mpool.tile([c.HCW, c.BC], F32, tag="pm2")
                for ki in range(c.nh):
                    nc.tensor.matmul(
                        out=pm2[:],
                        lhsT=wb16_sb[0:c.HCW,
                                     c.MW2 + (ki * 2 + hi) * 100:
                                     c.MW2 + (ki * 2 + hi) * 100 + 100],
                        rhs=o1_sb[:, ki, :],
                        start=(ki == 0), stop=(ki == c.nh - 1))
                nc.scalar.activation(
                    out=out_sb[:, hi, :], in_=pm2[:], func=AF.Relu,
                    bias=wb32_sb[0:c.HCW, 4 + hi:5 + hi], scale=1.0)
            for hi in range(c.nh):
                nc.sync.dma_start(out=out_d[hi], in_=out_sb[:, hi, :])

    nc.finalize()
    return nc


# ------------------------------------------------------------------ host


def wrap16(idx, n):
    """int16 index list -> [128, n/16] wrapped + replicated per Q7 core."""
    return np.ascontiguousarray(
        np.tile(idx.astype(np.int16).reshape(n // 16, 16).T, (8, 1)))


def host_prepare(cfg: Cfg, words, subj_pos, obj_pos, emb_table,
                 w1, b1, w2, b2, mw1, mb1, mw2, mb2):
    c = cfg
    words = np.asarray(words).astype(np.int64)
    subj_pos = np.asarray(subj_pos)
    obj_pos = np.asarray(obj_pos)
    f32 = lambda x: np.asarray(x, dtype=np.float32)

    table = np.zeros((c.V, c.E), BFNP)
    table[:, :c.D] = f32(emb_table).astype(BFNP)

    w1 = f32(w1)
    mw1 = f32(mw1)
    mw1e = np.concatenate([mw1[0:c.D] + mw1[c.D:2 * c.D],
                           mw1[2 * c.D:3 * c.D], mw1[3 * c.D:4 * c.D]], axis=0)
    mw2 = f32(mw2)
    w2v = f32(w2).reshape(c.H)

    def padD(m):  # [300, H] -> [384, H]
        out = np.zeros((c.DP, m.shape[1]), np.float32)
        out[:c.D] = m
        return out

    w1a = padD(w1[:c.D])
    w1b = padD(w1[c.D:2 * c.D])
    mw1p = np.concatenate([padD(mw1e[i * c.D:(i + 1) * c.D]) for i in range(3)],
                          axis=0)  # [1152, 200]

    wb16 = np.zeros((128, c.NC16), np.float32)
    for ci in range(3):
        wb16[:, c.W1A + ci * 200:c.W1A + (ci + 1) * 200] = w1a[ci * 128:(ci + 1) * 128]
        wb16[:, c.W1B + ci * 200:c.W1B + (ci + 1) * 200] = w1b[ci * 128:(ci + 1) * 128]
    for kk in range(9):
        wb16[:, c.MW1 + kk * 200:c.MW1 + (kk + 1) * 200] = \
            mw1p[kk * 128:(kk + 1) * 128]
    for ki in range(2):
        for hi in range(2):
            wb16[:c.HCW, c.MW2 + (ki * 2 + hi) * 100:
                 c.MW2 + (ki * 2 + hi) * 100 + 100] = \
                mw2[ki * 100:(ki + 1) * 100, hi * 100:(hi + 1) * 100]
    for hi in range(2):
        wb16[:c.HCW, c.W2 + hi] = w2v[hi * 100:(hi + 1) * 100]

    wb32 = np.zeros((128, 6), np.float32)
    wb32[:c.HCW, 0] = f32(b1)[:c.HCW]
    wb32[:c.HCW, 1] = f32(b1)[c.HCW:]
    wb32[:c.HCW, 2] = f32(mb1)[:c.HCW]
    wb32[:c.HCW, 3] = f32(mb1)[c.HCW:]
    wb32[:c.HCW, 4] = f32(mb2)[:c.HCW]
    wb32[:c.HCW, 5] = f32(mb2)[c.HCW:]

    in_maps = []
    for core in range(c.NCORES):
        b0 = core * c.BC
        lo_list, hi_list = [], []
        wb16c = wb16.copy()
        for b in range(c.BC):
            w = words[b0 + b]
            order = np.argsort(w, kind="stable")
            ws = w[order]
            if ws[c.HALF - 1] >= c.LO_MAX or ws[c.HALF] < c.HB:
                raise RuntimeError(
                    f"batch {b0 + b}: vocab split infeasible "
                    f"(lo_max={ws[c.HALF - 1]}, hi_min={ws[c.HALF]})")
            lo_list.append(ws[:c.HALF])
            hi_list.append(ws[c.HALF:] - c.HB)
            for m, pos in ((0, subj_pos), (1, obj_pos)):
                pm = (np.asarray(pos[b0 + b])[order] != 0)
                av = np.where(pm, np.float32(MASK_ADD), np.float32(0.0))
                # madd[p, si] at wb16 col MADD + (m*16+b)*4 + si
                mc = c.MADD + (m * c.BC + b) * 4
                wb16c[:, mc:mc + 4] = av.reshape(4, 128).T
        idx = np.concatenate(
            [wrap16(np.concatenate(lo_list), c.HALF * c.BC),
             wrap16(np.concatenate(hi_list), c.HALF * c.BC)], axis=1)
        in_maps.append({
            "idx": np.ascontiguousarray(idx),
            "wb16": np.ascontiguousarray(wb16c.astype(BFNP)),
            "wb32": wb32,
            "table": table,
        })
    return in_maps


def assemble_output(cfg: Cfg, results):
    c = cfg
    outs = []
    for core in range(c.NCORES):
        o = results[core]["out"]                      # [nh, HCW, BC]
        outs.append(np.transpose(o, (2, 0, 1)).reshape(c.BC, c.H))
    return np.ascontiguousarray(np.concatenate(outs, axis=0))


_CACHE = {}


def run(inputs, trace=False, **kw):
    from concourse.bass_utils import run_bass_kernel_spmd

    cfg = Cfg()
    in_maps = host_prepare(cfg, **{k: inputs[k] for k in (
        "words", "subj_pos", "obj_pos", "emb_table", "w1", "b1", "w2", "b2",
        "mw1", "mb1", "mw2", "mb2")})
    if "nc" not in _CACHE:
        _CACHE["nc"] = build_nc(cfg)
    nc = _CACHE["nc"]
    res = run_bass_kernel_spmd(nc, in_maps, core_ids=list(range(cfg.NCORES)),
                               trace=trace, **kw)
    return assemble_output(cfg, res.results), res


def kernel(**inputs) -> np.ndarray:
    return run(inputs)[0]


# revision 10
# speedup vs baseline: 1.2186x; 1.1144x over previous
"""Trainium2 Bass kernel for nn_AttentionNet (pooling / ridge regime).

Model (per batch b of B=128, L=512, D=300, H=200, V=50000):
  word_emb = emb_table[words]                          [B,L,D]
  subj_emb = max over l with subj_pos[b,l]==0 of word_emb (else -1e12)
  obj_emb  = same with obj_pos
  hid  = tanh(word_emb @ w1[:D] + subj_emb @ w1[D:] + b1)
  attn = softmax(hid @ w2, axis=l)    (b2 dropped: softmax shift-invariant)
  subj_attn = sum_l attn * word_emb   (obj_attn identical -- source bug)
  out = relu(relu(cat([subj_attn, subj_attn, subj_emb, obj_emb]) @ mw1 + mb1) @ mw2 + mb2)

Sharding: pure data parallel, 16 batches per core on 8 cores; embedding
table (bf16) and the small weights replicated.

Device plan per core (16 batches = 16 token-tiles of 512):
  - everything on-chip is bf16 (weights, embeddings) with fp32 PSUM /
    softmax / biases.  Table rows are padded to 384 bf16 columns (768 B,
    the 256B gather granularity); pad values are 0.0 and meet zero
    weight rows everywhere downstream.
  - bulk gather via gpsimd.dma_gather (int16 indices, vocab sorted/split
    per batch into low/high 32768-row windows).  Gathers are issued
    first (shared num_idxs register, identity shipped in the weight blob
    so no gpsimd ucode-library churn) and compute chases gather pairs
    group by group.
  - all weights + madd masks + identity ship in one packed bf16 blob +
    one small f32 bias blob (two DMAs total).
  - masked max pools: fused (emb+mask) max chains via
    scalar_tensor_tensor on DVE (300-wide), 3 PE transposes, PSUM
    reduce.
  - per group of 4 batches: PE-transpose embeddings to D-major, dense
    hid/score matmuls, per-group softmax + attention-weighted sum so the
    attention tail pipelines with the next group's matmul stream.
"""

import numpy as np
import ml_dtypes

import concourse.bass as bass
import concourse.bacc as bacc
import concourse.mybir as mybir
import concourse.tile as tile
from contextlib import ExitStack

F32 = mybir.dt.float32
BF16 = mybir.dt.bfloat16
I16 = mybir.dt.int16

NEG_INF = 1e12      # reference constant
MASK_ADD = -2e12    # additive mask; clamped back to -NEG_INF at the end
BFNP = ml_dtypes.bfloat16

AX = mybir.AxisListType.X
MAX = mybir.AluOpType.max
ADD = mybir.AluOpType.add
AF = mybir.ActivationFunctionType


class Cfg:
    def __init__(self):
        self.B, self.L, self.D, self.H, self.V = 128, 512, 300, 200, 50000
        self.NCORES = 8
        self.BC = 16                  # batches per core
        self.PT = 128                 # tokens per subtile
        self.NSUB = 4                 # subtiles per batch
        self.NS = 64                  # subtiles per core
        self.DP = 384                 # padded D (3 chunks of 128)
        self.E = 384                  # gather row elems (bf16) = 768 B
        self.HCW = 100                # H chunk width
        self.nh = 2
        self.nd = 3
        self.GB = 4                   # batches per group
        self.NG = 4
        self.HALF = 256               # tokens per vocab half
        self.LO_MAX = 32768
        self.HB = self.V - 32768      # high window base
        self.nsp = 4                  # gather splits per half
        self.dch = [(0, 128), (128, 128), (256, 128)]  # D chunks (padded)
        # wb16 blob column offsets
        self.W1A = 0
        self.W1B = 600
        self.MW1 = 1200
        self.MW2 = 3000
        self.W2 = 3400
        self.IDB = 3402
        self.NC16 = self.IDB + 128
        self.MADD = 6                 # wb32 col offset
        self.NC32 = 6 + 128

    def subtiles(self, b):
        return [2 * b, 2 * b + 1, 32 + 2 * b, 32 + 2 * b + 1]


def build_nc(cfg: Cfg):
    c = cfg
    nc = bacc.Bacc(num_swdge_queues=4)

    idx_d = nc.declare_dram_parameter("idx", [128, 512], I16, isOutput=False)
    wb16_d = nc.declare_dram_parameter("wb16", [128, c.NC16], BF16, isOutput=False)
    wb32_d = nc.declare_dram_parameter("wb32", [128, c.NC32], F32, isOutput=False)
    table = nc.declare_dram_parameter("table", [c.V, c.E], BF16, isOutput=False)
    out_d = nc.declare_dram_parameter("out", [c.nh, c.HCW, c.BC], F32, isOutput=True)

    with tile.TileContext(nc) as tc, ExitStack() as ctx:
        sb = ctx.enter_context(tc.tile_pool(name="sb", bufs=1))

        ix_sb = sb.tile([128, 512], I16)
        wb16_sb = sb.tile([128, c.NC16], BF16)
        wb32_sb = sb.tile([128, c.NC32], F32)
        emb_tok = sb.tile([128, c.NS, c.E], BF16)
        pooledf = sb.tile([128, 2, c.nd, c.BC], BF16)
        pooled = sb.tile([128, 2, c.nd, c.BC], BF16)
        bias_sb = sb.tile([c.HCW, c.nh, c.BC], F32)
        attn_t = sb.tile([128, c.NSUB, c.BC], BF16)
        sattn_sb = sb.tile([c.BC, c.E], BF16)
        satd = sb.tile([128, c.nd, c.BC], BF16)
        o1_sb = sb.tile([c.HCW, c.nh, c.BC], BF16)
        out_sb = sb.tile([c.HCW, c.nh, c.BC], F32)

        identb = wb16_sb[:, c.IDB:c.IDB + 128]

        # index DMA first: it gates gather descriptor generation
        nc.sync.dma_start(out=ix_sb[:], in_=idx_d[:])
        nc.sync.dma_start(out=wb16_sb[:], in_=wb16_d[:])
        nc.sync.dma_start(out=wb32_sb[:], in_=wb32_d[:])

        # gathers: lo/hi pair per split k; pair k feeds batch group k
        npc = c.HALF * c.BC // c.nsp          # 1024 rows per gather
        nsubt = npc // 128                    # 8 subtiles per gather
        nreg = nc.gpsimd.to_reg(npc)
        for k in range(c.nsp):
            i0 = k * (npc // 16)
            nc.gpsimd.dma_gather(
                out_ap=emb_tok[:, k * nsubt:(k + 1) * nsubt, :],
                in_ap=table[0:c.LO_MAX, :],
                idxs_ap=ix_sb[:, i0:i0 + npc // 16],
                num_idxs=npc, num_idxs_reg=nreg, elem_size=c.E,
                single_packet=False, queue_num=(2 * k) % 4)
            nc.gpsimd.dma_gather(
                out_ap=emb_tok[:, 32 + k * nsubt:32 + (k + 1) * nsubt, :],
                in_ap=table[c.HB:c.V, :],
                idxs_ap=ix_sb[:, 256 + i0:256 + i0 + npc // 16],
                num_idxs=npc, num_idxs_reg=nreg, elem_size=c.E,
                single_packet=False, queue_num=(2 * k + 1) % 4)

        def w16(col, n, p=128):
            return wb16_sb[0:p, col:col + n]

        with tc.tile_pool(name="edpool", bufs=8) as edpool, \
             tc.tile_pool(name="mkpool", bufs=4) as mkpool, \
             tc.tile_pool(name="hidpool", bufs=2) as hidpool, \
             tc.tile_pool(name="sstpool", bufs=2) as sstpool, \
             tc.tile_pool(name="saspool", bufs=2) as saspool, \
             tc.tile_pool(name="scgpool", bufs=2) as scgpool, \
             tc.tile_pool(name="smpool", bufs=2) as smpool, \
             tc.tile_pool(name="tepool", bufs=2, space="PSUM") as tepool, \
             tc.tile_pool(name="tppool", bufs=2, space="PSUM") as tppool, \
             tc.tile_pool(name="hpool", bufs=2, space="PSUM") as hpool, \
             tc.tile_pool(name="spool", bufs=1, space="PSUM") as spool:
            for g in range(c.NG):
                g0 = g * c.GB
                gsl = slice(g0, g0 + c.GB)
                eds = []
                for b in range(g0, g0 + c.GB):
                    subs = c.subtiles(b)
                    ed = edpool.tile([128, c.nd, c.L], BF16, tag="ed")
                    for ci, (d0, dn) in enumerate(c.dch):
                        te = tepool.tile([128, c.NSUB, 128], BF16, tag="te")
                        for si, s in enumerate(subs):
                            nc.tensor.transpose(
                                out=te[0:dn, si, :],
                                in_=emb_tok[:, s, d0:d0 + dn],
                                identity=identb)
                        if ci == 0 or (ci == 2 and b % 2 == 0):
                            nc.scalar.copy(out=ed[0:dn, ci, :], in_=te[0:dn, :, :])
                        else:
                            nc.vector.tensor_copy(out=ed[0:dn, ci, :], in_=te[0:dn, :, :])
                    eds.append(ed)
                    # masked max pools: fused (emb+mask) max chain on DVE
                    for m in range(2):
                        mcol = c.MADD + (m * c.BC + b) * 4
                        mk = mkpool.tile([128, c.E], BF16, tag="mk")
                        nc.vector.tensor_scalar_add(
                            out=mk[:], in0=emb_tok[:, subs[0], :],
                            scalar1=wb32_sb[:, mcol:mcol + 1])
                        for si in range(1, 4):
                            nc.vector.scalar_tensor_tensor(
                                out=mk[:], in0=emb_tok[:, subs[si], :],
                                scalar=wb32_sb[:, mcol + si:mcol + si + 1],
                                in1=mk[:], op0=ADD, op1=MAX)
                        tp = tppool.tile([128, c.nd, 128], BF16, tag="tp")
                        for ci, (d0, dn) in enumerate(c.dch):
                            nc.tensor.transpose(
                                out=tp[0:dn, ci, :],
                                in_=mk[:, d0:d0 + dn],
                                identity=identb)
                        nc.vector.tensor_reduce(
                            out=pooledf[:, m, :, b], in_=tp[:],
                            axis=AX, op=MAX)
                # clamp restores exact -1e12 for all-masked rows
                nc.any.tensor_scalar_max(
                    out=pooled[:, :, :, gsl], in0=pooledf[:, :, :, gsl],
                    scalar1=-NEG_INF)
                # tanh bias for the group: w1b^T subj_emb + b1
                for hi in range(c.nh):
                    pb = spool.tile([c.HCW, c.GB], F32, tag="pb")
                    for ci in range(c.nd):
                        nc.tensor.matmul(
                            out=pb[:],
                            lhsT=w16(c.W1B + ci * 200 + hi * 100, 100),
                            rhs=pooled[:, 0, ci, gsl],
                            start=(ci == 0), stop=(ci == c.nd - 1))
                    nc.scalar.activation(
                        out=bias_sb[:, hi, gsl], in_=pb[:],
                        func=AF.Identity, bias=wb32_sb[0:c.HCW, hi:hi + 1],
                        scale=1.0)
                # dense hid + scores per batch
                sst = sstpool.tile([1, c.GB, c.L], F32, tag="sst")
                for bi, b in enumerate(range(g0, g0 + c.GB)):
                    ed = eds[bi]
                    hid = hidpool.tile([c.HCW, c.nh, c.L], BF16, tag="hid")
                    for hi in range(c.nh):
                        ph = hpool.tile([c.HCW, c.L], F32, tag="ph")
                        for ci in range(c.nd):
                            nc.tensor.matmul(
                                out=ph[:],
                                lhsT=w16(c.W1A + ci * 200 + hi * 100, 100),
                                rhs=ed[:, ci, :],
                                start=(ci == 0), stop=(ci == c.nd - 1))
                        nc.scalar.activation(
                            out=hid[:, hi, :], in_=ph[:],
                            func=AF.Tanh, bias=bias_sb[:, hi, b:b + 1],
                            scale=1.0)
                    ps = spool.tile([1, c.L], F32, tag="ps")
                    for hi in range(c.nh):
                        nc.tensor.matmul(
                            out=ps[:],
                            lhsT=wb16_sb[0:c.HCW, c.W2 + hi:c.W2 + hi + 1],
                            rhs=hid[:, hi, :],
                            start=(hi == 0), stop=(hi == c.nh - 1))
                    if bi % 2 == 0:
                        nc.vector.tensor_copy(out=sst[0:1, bi, :], in_=ps[:])
                    else:
                        nc.scalar.copy(out=sst[0:1, bi, :], in_=ps[:])
                # batch-major softmax for the group
                scg = scgpool.tile([c.GB, c.L], F32, tag="sc")
                nc.sync.dma_start(out=scg[:], in_=sst[:])
                sm = smpool.tile([c.GB, 4], F32, tag="sm")
                nc.vector.tensor_reduce(out=sm[:, 0:1], in_=scg[:], axis=AX, op=MAX)
                nc.any.tensor_scalar_mul(out=sm[:, 1:2], in0=sm[:, 0:1], scalar1=-1.0)
                ax = scgpool.tile([c.GB, c.L], F32, tag="ax")
                nc.scalar.activation(out=ax[:], in_=scg[:], func=AF.Exp,
                                     bias=sm[:, 1:2], scale=1.0)
                nc.vector.tensor_reduce(out=sm[:, 2:3], in_=ax[:], axis=AX, op=ADD)
                nc.vector.reciprocal(out=sm[:, 3:4], in_=sm[:, 2:3])
                atb = scgpool.tile([c.GB, c.L], BF16, tag="atb")
                nc.any.tensor_scalar_mul(out=atb[:], in0=ax[:], scalar1=sm[:, 3:4])
                for si in range(c.NSUB):
                    tp = tppool.tile([128, c.nd, 128], BF16, tag="tp")
                    nc.tensor.transpose(
                        out=tp[:, 0, 0:c.GB],
                        in_=atb[:, si * 128:(si + 1) * 128],
                        identity=wb16_sb[0:c.GB, c.IDB:c.IDB + c.GB])
                    nc.any.tensor_copy(out=attn_t[:, si, gsl], in_=tp[:, 0, 0:c.GB])
                # attention-weighted sum
                sas = saspool.tile([1, c.GB, c.E], BF16, tag="sas")
                for bi, b in enumerate(range(g0, g0 + c.GB)):
                    pw = spool.tile([1, c.L], F32, tag="ps")
                    for si, s in enumerate(c.subtiles(b)):
                        nc.tensor.matmul(
                            out=pw[:, 0:c.E],
                            lhsT=attn_t[:, si, b:b + 1],
                            rhs=emb_tok[:, s, :],
                            start=(si == 0), stop=(si == c.NSUB - 1))
                    if bi % 2 == 0:
                        nc.vector.tensor_copy(out=sas[0:1, bi, :], in_=pw[:, 0:c.E])
                    else:
                        nc.scalar.copy(out=sas[0:1, bi, :], in_=pw[:, 0:c.E])
                nc.sync.dma_start(out=sattn_sb[gsl, :], in_=sas[:])

        # ---- tail: satd transposes + output MLP
        with tc.tile_pool(name="stpool", bufs=1, space="PSUM") as stpool, \
             tc.tile_pool(name="mpool", bufs=2, space="PSUM") as mpool:
            for ci, (d0, dn) in enumerate(c.dch):
                pst = stpool.tile([128, c.BC], BF16, tag="pst")
                nc.tensor.transpose(
                    out=pst[0:dn, :],
                    in_=sattn_sb[:, d0:d0 + dn],
                    identity=wb16_sb[0:c.BC, c.IDB:c.IDB + c.BC])
                nc.any.tensor_copy(out=satd[0:dn, ci, :], in_=pst[0:dn, :])
            for hi in range(c.nh):
                pm = mpool.tile([c.HCW, c.BC], F32, tag="pm")
                k = 0
                for blk in range(3):
                    for ci in range(c.nd):
                        rhs = satd[:, ci, :] if blk == 0 else pooled[:, blk - 1, ci, :]
                        nc.tensor.matmul(
                            out=pm[:],
                            lhsT=w16(c.MW1 + (blk * 3 + ci) * 200 + hi * 100, 100),
                            rhs=rhs, start=(k == 0), stop=(k == 8))
                        k += 1
                nc.scalar.activation(
                    out=o1_sb[:, hi, :], in_=pm[:], func=AF.Relu,
                    bias=wb32_sb[0:c.HCW, 2 + hi:3 + hi], scale=1.0)
            for hi in range(c.nh):
                pm2 = mpool.tile([c.HCW, c.BC], F32, tag="pm2")
                for ki in range(c.nh):
                    nc.tensor.matmul(
                        out=pm2[:],
                        lhsT=wb16_sb[0:c.HCW,
                                     c.MW2 + (ki * 2 + hi) * 100:
                                     c.MW2 + (ki * 2 + hi) * 100 + 100],
                        rhs=o1_sb[:, ki, :],
                        start=(ki == 0), stop=(ki == c.nh - 1))
                nc.scalar.activation(
                    out=out_sb[:, hi, :], in_=pm2[:], func=AF.Relu,
                    bias=wb32_sb[0:c.HCW, 4 + hi:5 + hi], scale=1.0)
            for hi in range(c.nh):
                nc.sync.dma_start(out=out_d[hi], in_=out_sb[:, hi, :])

    nc.finalize()
    return nc


# ------------------------------------------------------------------ host


def wrap16(idx, n):
    """int16 index list -> [128, n/16] wrapped + replicated per Q7 core."""
    return np.ascontiguousarray(
        np.tile(idx.astype(np.int16).reshape(n // 16, 16).T, (8, 1)))


def host_prepare(cfg: Cfg, words, subj_pos, obj_pos, emb_table,
                 w1, b1, w2, b2, mw1, mb1, mw2, mb2):
    c = cfg
    words = np.asarray(words).astype(np.int64)
    subj_pos = np.asarray(subj_pos)
    obj_pos = np.asarray(obj_pos)
    f32 = lambda x: np.asarray(x, dtype=np.float32)

    table = np.zeros((c.V, c.E), BFNP)
    table[:, :c.D] = f32(emb_table).astype(BFNP)

    w1 = f32(w1)
    mw1 = f32(mw1)
    mw1e = np.concatenate([mw1[0:c.D] + mw1[c.D:2 * c.D],
                           mw1[2 * c.D:3 * c.D], mw1[3 * c.D:4 * c.D]], axis=0)
    mw2 = f32(mw2)
    w2v = f32(w2).reshape(c.H)

    def padD(m):  # [300, H] -> [384, H]
        out = np.zeros((c.DP, m.shape[1]), np.float32)
        out[:c.D] = m
        return out

    w1a = padD(w1[:c.D])
    w1b = padD(w1[c.D:2 * c.D])
    mw1p = np.concatenate([padD(mw1e[i * c.D:(i + 1) * c.D]) for i in range(3)],
                          axis=0)  # [1152, 200]

    wb16 = np.zeros((128, c.NC16), np.float32)
    for ci in range(3):
        wb16[:, c.W1A + ci * 200:c.W1A + (ci + 1) * 200] = w1a[ci * 128:(ci + 1) * 128]
        wb16[:, c.W1B + ci * 200:c.W1B + (ci + 1) * 200] = w1b[ci * 128:(ci + 1) * 128]
    for kk in range(9):
        wb16[:, c.MW1 + kk * 200:c.MW1 + (kk + 1) * 200] = \
            mw1p[kk * 128:(kk + 1) * 128]
    for ki in range(2):
        for hi in range(2):
            wb16[:c.HCW, c.MW2 + (ki * 2 + hi) * 100:
                 c.MW2 + (ki * 2 + hi) * 100 + 100] = \
                mw2[ki * 100:(ki + 1) * 100, hi * 100:(hi + 1) * 100]
    for hi in range(2):
        wb16[:c.HCW, c.W2 + hi] = w2v[hi * 100:(hi + 1) * 100]
    wb16[:, c.IDB:c.IDB + 128] = np.eye(128, dtype=np.float32)

    wb32 = np.zeros((128, c.NC32), np.float32)
    wb32[:c.HCW, 0] = f32(b1)[:c.HCW]
    wb32[:c.HCW, 1] = f32(b1)[c.HCW:]
    wb32[:c.HCW, 2] = f32(mb1)[:c.HCW]
    wb32[:c.HCW, 3] = f32(mb1)[c.HCW:]
    wb32[:c.HCW, 4] = f32(mb2)[:c.HCW]
    wb32[:c.HCW, 5] = f32(mb2)[c.HCW:]

    wb16b = np.ascontiguousarray(wb16.astype(BFNP))
    in_maps = []
    for core in range(c.NCORES):
        b0 = core * c.BC
        lo_list, hi_list = [], []
        wb32c = wb32.copy()
        for b in range(c.BC):
            w = words[b0 + b]
            order = np.argsort(w, kind="stable")
            ws = w[order]
            if ws[c.HALF - 1] >= c.LO_MAX or ws[c.HALF] < c.HB:
                raise RuntimeError(
                    f"batch {b0 + b}: vocab split infeasible "
                    f"(lo_max={ws[c.HALF - 1]}, hi_min={ws[c.HALF]})")
            lo_list.append(ws[:c.HALF])
            hi_list.append(ws[c.HALF:] - c.HB)
            for m, pos in ((0, subj_pos), (1, obj_pos)):
                pm = (np.asarray(pos[b0 + b])[order] != 0)
                av = np.where(pm, np.float32(MASK_ADD), np.float32(0.0))
                # madd[p, si] at wb32 col MADD + (m*16+b)*4 + si
                mc = c.MADD + (m * c.BC + b) * 4
                wb32c[:, mc:mc + 4] = av.reshape(4, 128).T
        idx = np.concatenate(
            [wrap16(np.concatenate(lo_list), c.HALF * c.BC),
             wrap16(np.concatenate(hi_list), c.HALF * c.BC)], axis=1)
        in_maps.append({
            "idx": np.ascontiguousarray(idx),
            "wb16": wb16b,
            "wb32": wb32c,
            "table": table,
        })
    return in_maps


def assemble_output(cfg: Cfg, results):
    c = cfg
    outs = []
    for core in range(c.NCORES):
        o = results[core]["out"]                      # [nh, HCW, BC]
        outs.append(np.transpose(o, (2, 0, 1)).reshape(c.BC, c.H))
    return np.ascontiguousarray(np.concatenate(outs, axis=0))


_CACHE = {}


def run(inputs, trace=False, **kw):
    from concourse.bass_utils import run_bass_kernel_spmd

    cfg = Cfg()
    in_maps = host_prepare(cfg, **{k: inputs[k] for k in (
        "words", "subj_pos", "obj_pos", "emb_table", "w1", "b1", "w2", "b2",
        "mw1", "mb1", "mw2", "mb2")})
    if "nc" not in _CACHE:
        _CACHE["nc"] = build_nc(cfg)
    nc = _CACHE["nc"]
    res = run_bass_kernel_spmd(nc, in_maps, core_ids=list(range(cfg.NCORES)),
                               trace=trace, **kw)
    return assemble_output(cfg, res.results), res


def kernel(**inputs) -> np.ndarray:
    return run(inputs)[0]


# revision 11
# speedup vs baseline: 1.4666x; 1.2035x over previous
"""Trainium2 Bass kernel for nn_AttentionNet (pooling / ridge regime).

Model (per batch b of B=128, L=512, D=300, H=200, V=50000):
  word_emb = emb_table[words]                          [B,L,D]
  subj_emb = max over l with subj_pos[b,l]==0 of word_emb (else -1e12)
  obj_emb  = same with obj_pos
  hid  = tanh(word_emb @ w1[:D] + subj_emb @ w1[D:] + b1)
  attn = softmax(hid @ w2, axis=l)    (b2 dropped: softmax shift-invariant)
  subj_attn = sum_l attn * word_emb   (obj_attn identical -- source bug)
  out = relu(relu(cat([subj_attn, subj_attn, subj_emb, obj_emb]) @ mw1 + mb1) @ mw2 + mb2)

Sharding: pure data parallel, 16 batches per core on 8 cores; embedding
table (bf16) and the small weights replicated.

Device plan per core (16 batches, bf16 everywhere, fp32 PSUM/biases):
  - table rows padded to 384 bf16 cols (768 B = the 256B gather
    granularity); pad values 0.0 meet zero weight rows downstream.
  - the masked max-pools are precomputed as GATHERED PARTICIPANT
    SUBTILES: the host collects, per (batch, mask, vocab-window), the
    <=64 tokens that participate in the pool (pos==0) and pads them to
    64 slots by repeating the first participant (max unchanged).  A
    batch PAIR's two 64-slot sets form one 128-row subtile.  The pool is
    then 3 PE transposes + a half-range PSUM max-reduce per subtile --
    no mask tensors, no mask arithmetic anywhere.
  - scores exp is fused into the PSUM->SBUF score copy (softmax without
    max-shift: |score| <= ~10 so exp is safe in bf16/f32), normalization
    happens batch-major after one small DMA.
  - gathers are issued first (shared num_idxs register, identity shipped
    inside the weight blob so gpsimd never switches ucode libraries for
    compute) and the per-group compute chases gather pairs.
  - per group of 4 batches: PE-transpose embeddings to D-major, dense
    hid/score matmuls, per-group softmax + attention-weighted sum so the
    attention tail pipelines with the next group's matmul stream.
"""

import numpy as np
import ml_dtypes

import concourse.bass as bass
import concourse.bacc as bacc
import concourse.mybir as mybir
import concourse.tile as tile
from contextlib import ExitStack

F32 = mybir.dt.float32
BF16 = mybir.dt.bfloat16
I16 = mybir.dt.int16

NEG_INF = 1e12      # reference constant
BFNP = ml_dtypes.bfloat16

AX = mybir.AxisListType.X
MAX = mybir.AluOpType.max
ADD = mybir.AluOpType.add
AF = mybir.ActivationFunctionType


class Cfg:
    def __init__(self):
        self.B, self.L, self.D, self.H, self.V = 128, 512, 300, 200, 50000
        self.NCORES = 8
        self.BC = 16                  # batches per core
        self.NSUB = 4                 # token subtiles per batch
        self.DP = 384                 # padded D (3 chunks of 128)
        self.E = 384                  # gather row elems (bf16) = 768 B
        self.HCW = 100                # H chunk width
        self.nh = 2
        self.nd = 3
        self.GB = 4                   # batches per group
        self.NG = 4
        self.HALF = 256               # tokens per vocab half
        self.LO_MAX = 32768
        self.HB = self.V - 32768      # high window base
        self.nsp = 4                  # gather splits per half
        self.PCAP = 64                # participant slots per (batch, window)
        # per half-split: 8 batch subtiles + 4 participant subtiles
        self.SPS = 12
        self.NS = 2 * self.nsp * self.SPS     # 96 subtiles
        self.NPC = 128 * self.SPS             # 1536 rows per gather
        self.dch = [(0, 128), (128, 128), (256, 128)]
        # wb16 blob column offsets
        self.W1A = 0
        self.W1B = 600
        self.MW1 = 1200
        self.MW2 = 3000
        self.W2 = 3400
        self.IDB = 3402
        self.NC16 = self.IDB + 128

    def sb_lo(self, b):
        return 12 * (b // 4) + 2 * (b % 4)

    def subtiles(self, b):
        lo = self.sb_lo(b)
        return [lo, lo + 1, 48 + lo, 48 + lo + 1]

    def pp(self, p, m, w):
        """Participant subtile for batch pair p, mask m, window w."""
        return (0 if w == 0 else 48) + 12 * (p // 2) + 8 + 2 * (p % 2) + m


def build_nc(cfg: Cfg):
    c = cfg
    nc = bacc.Bacc(num_swdge_queues=4)

    nix = c.NPC * c.nsp // 16                 # idx cols per half
    idx_d = nc.declare_dram_parameter("idx", [128, 2 * nix], I16, isOutput=False)
    wb16_d = nc.declare_dram_parameter("wb16", [128, c.NC16], BF16, isOutput=False)
    wb32_d = nc.declare_dram_parameter("wb32", [128, 6], F32, isOutput=False)
    table = nc.declare_dram_parameter("table", [c.V, c.E], BF16, isOutput=False)
    out_d = nc.declare_dram_parameter("out", [c.nh, c.HCW, c.BC], F32, isOutput=True)

    with tile.TileContext(nc) as tc, ExitStack() as ctx:
        sb = ctx.enter_context(tc.tile_pool(name="sb", bufs=1))

        ix_sb = sb.tile([128, 2 * nix], I16)
        wb16_sb = sb.tile([128, c.NC16], BF16)
        wb32_sb = sb.tile([128, 6], F32)
        emb_tok = sb.tile([128, c.NS, c.E], BF16)
        pooledf = sb.tile([128, 2, c.nd, c.BC], BF16)
        pooled2 = sb.tile([128, 2, c.nd, c.BC], BF16)
        pooled = sb.tile([128, 2, c.nd, c.BC], BF16)
        bias_sb = sb.tile([c.HCW, c.nh, c.BC], F32)
        attn_t = sb.tile([128, c.NSUB, c.BC], BF16)
        sattn_sb = sb.tile([c.BC, c.E], BF16)
        satd = sb.tile([128, c.nd, c.BC], BF16)
        o1_sb = sb.tile([c.HCW, c.nh, c.BC], BF16)
        out_sb = sb.tile([c.HCW, c.nh, c.BC], F32)

        identb = wb16_sb[:, c.IDB:c.IDB + 128]

        # index DMA first: it gates gather descriptor generation
        nc.sync.dma_start(out=ix_sb[:], in_=idx_d[:])
        nc.sync.dma_start(out=wb16_sb[:], in_=wb16_d[:])
        nc.sync.dma_start(out=wb32_sb[:], in_=wb32_d[:])

        # gathers: lo/hi pair per split k; pair k feeds batch group k
        npc = c.NPC
        nreg = nc.gpsimd.to_reg(npc)
        for k in range(c.nsp):
            i0 = k * (npc // 16)
            nc.gpsimd.dma_gather(
                out_ap=emb_tok[:, k * c.SPS:(k + 1) * c.SPS, :],
                in_ap=table[0:c.LO_MAX, :],
                idxs_ap=ix_sb[:, i0:i0 + npc // 16],
                num_idxs=npc, num_idxs_reg=nreg, elem_size=c.E,
                single_packet=False, queue_num=(2 * k) % 4)
            nc.gpsimd.dma_gather(
                out_ap=emb_tok[:, 48 + k * c.SPS:48 + (k + 1) * c.SPS, :],
                in_ap=table[c.HB:c.V, :],
                idxs_ap=ix_sb[:, nix + i0:nix + i0 + npc // 16],
                num_idxs=npc, num_idxs_reg=nreg, elem_size=c.E,
                single_packet=False, queue_num=(2 * k + 1) % 4)

        def w16(col, n, p=128):
            return wb16_sb[0:p, col:col + n]

        with tc.tile_pool(name="edpool", bufs=8) as edpool, \
             tc.tile_pool(name="hidpool", bufs=2) as hidpool, \
             tc.tile_pool(name="sstpool", bufs=2) as sstpool, \
             tc.tile_pool(name="saspool", bufs=2) as saspool, \
             tc.tile_pool(name="scgpool", bufs=2) as scgpool, \
             tc.tile_pool(name="smpool", bufs=2) as smpool, \
             tc.tile_pool(name="tepool", bufs=2, space="PSUM") as tepool, \
             tc.tile_pool(name="tppool", bufs=2, space="PSUM") as tppool, \
             tc.tile_pool(name="hpool", bufs=2, space="PSUM") as hpool, \
             tc.tile_pool(name="spool", bufs=1, space="PSUM") as spool, \
             tc.tile_pool(name="wpool", bufs=1, space="PSUM") as wpool:
            for g in range(c.NG):
                g0 = g * c.GB
                gsl = slice(g0, g0 + c.GB)
                eds = []
                for b in range(g0, g0 + c.GB):
                    subs = c.subtiles(b)
                    ed = edpool.tile([128, c.nd, c.L], BF16, tag="ed")
                    for ci, (d0, dn) in enumerate(c.dch):
                        te = tepool.tile([128, c.NSUB, 128], BF16, tag="te")
                        for si, s in enumerate(subs):
                            nc.tensor.transpose(
                                out=te[:, si, :],
                                in_=emb_tok[:, s, d0:d0 + 128],
                                identity=identb)
                        if ci == 0 or (ci == 2 and b % 2 == 0):
                            nc.scalar.copy(out=ed[:, ci, :], in_=te[:])
                        else:
                            nc.vector.tensor_copy(out=ed[:, ci, :], in_=te[:])
                    eds.append(ed)
                # pools from participant subtiles (batch pairs 2g, 2g+1)
                for pi in range(2):
                    p = 2 * g + pi
                    for m in range(2):
                        for w, dst in ((0, pooledf), (1, pooled2)):
                            tp = tppool.tile([128, c.nd, 128], BF16, tag="tp")
                            s = c.pp(p, m, w)
                            for ci, (d0, dn) in enumerate(c.dch):
                                nc.tensor.transpose(
                                    out=tp[:, ci, :],
                                    in_=emb_tok[:, s, d0:d0 + 128],
                                    identity=identb)
                            nc.vector.tensor_reduce(
                                out=dst[:, m, :, 2 * p], in_=tp[:, :, 0:c.PCAP],
                                axis=AX, op=MAX)
                            nc.vector.tensor_reduce(
                                out=dst[:, m, :, 2 * p + 1],
                                in_=tp[:, :, c.PCAP:128],
                                axis=AX, op=MAX)
                # combine windows (clamp unnecessary: participants are real)
                nc.any.tensor_tensor(
                    out=pooled[:, :, :, gsl], in0=pooledf[:, :, :, gsl],
                    in1=pooled2[:, :, :, gsl], op=MAX)
                # tanh bias for the group: w1b^T subj_emb + b1
                for hi in range(c.nh):
                    pb = hpool.tile([c.HCW, c.L], F32, tag="ph")
                    for ci in range(c.nd):
                        nc.tensor.matmul(
                            out=pb[:, 0:c.GB],
                            lhsT=w16(c.W1B + ci * 200 + hi * 100, 100),
                            rhs=pooled[:, 0, ci, gsl],
                            start=(ci == 0), stop=(ci == c.nd - 1))
                    nc.scalar.activation(
                        out=bias_sb[:, hi, gsl], in_=pb[:, 0:c.GB],
                        func=AF.Identity, bias=wb32_sb[0:c.HCW, hi:hi + 1],
                        scale=1.0)
                # dense hid + scores per batch; exp fused into score copy
                sst = sstpool.tile([1, c.GB, c.L], BF16, tag="sst")
                for bi, b in enumerate(range(g0, g0 + c.GB)):
                    ed = eds[bi]
                    hid = hidpool.tile([c.HCW, c.nh, c.L], BF16, tag="hid")
                    for hi in range(c.nh):
                        ph = hpool.tile([c.HCW, c.L], F32, tag="ph")
                        for ci in range(c.nd):
                            nc.tensor.matmul(
                                out=ph[:],
                                lhsT=w16(c.W1A + ci * 200 + hi * 100, 100),
                                rhs=ed[:, ci, :],
                                start=(ci == 0), stop=(ci == c.nd - 1))
                        nc.scalar.activation(
                            out=hid[:, hi, :], in_=ph[:],
                            func=AF.Tanh, bias=bias_sb[:, hi, b:b + 1],
                            scale=1.0)
                    ps = spool.tile([1, c.L], F32, tag="ps")
                    for hi in range(c.nh):
                        nc.tensor.matmul(
                            out=ps[:],
                            lhsT=wb16_sb[0:c.HCW, c.W2 + hi:c.W2 + hi + 1],
                            rhs=hid[:, hi, :],
                            start=(hi == 0), stop=(hi == c.nh - 1))
                    nc.scalar.activation(out=sst[0:1, bi, :], in_=ps[:],
                                         func=AF.Exp)
                # batch-major normalization for the group
                scg = scgpool.tile([c.GB, c.L], BF16, tag="sc")
                nc.sync.dma_start(out=scg[:], in_=sst[:])
                sm = smpool.tile([c.GB, 2], F32, tag="sm")
                nc.vector.tensor_reduce(out=sm[:, 0:1], in_=scg[:], axis=AX, op=ADD)
                nc.vector.reciprocal(out=sm[:, 1:2], in_=sm[:, 0:1])
                atb = scgpool.tile([c.GB, c.L], BF16, tag="atb")
                nc.any.tensor_scalar_mul(out=atb[:], in0=scg[:], scalar1=sm[:, 1:2])
                for si in range(c.NSUB):
                    tp = tppool.tile([128, c.nd, 128], BF16, tag="tp")
                    nc.tensor.transpose(
                        out=tp[:, 0, 0:c.GB],
                        in_=atb[:, si * 128:(si + 1) * 128],
                        identity=wb16_sb[0:c.GB, c.IDB:c.IDB + c.GB])
                    nc.any.tensor_copy(out=attn_t[:, si, gsl], in_=tp[:, 0, 0:c.GB])
                # attention-weighted sum
                sas = saspool.tile([1, c.GB, c.E], BF16, tag="sas")
                for bi, b in enumerate(range(g0, g0 + c.GB)):
                    pw = wpool.tile([1, c.E], F32, tag="pw")
                    for si, s in enumerate(c.subtiles(b)):
                        nc.tensor.matmul(
                            out=pw[:],
                            lhsT=attn_t[:, si, b:b + 1],
                            rhs=emb_tok[:, s, :],
                            start=(si == 0), stop=(si == c.NSUB - 1))
                    if bi % 2 == 0:
                        nc.vector.tensor_copy(out=sas[0:1, bi, :], in_=pw[:])
                    else:
                        nc.scalar.copy(out=sas[0:1, bi, :], in_=pw[:])
                nc.scalar.dma_start(out=sattn_sb[gsl, :], in_=sas[:])

        # ---- tail: satd transposes + output MLP
        with tc.tile_pool(name="stpool", bufs=1, space="PSUM") as stpool, \
             tc.tile_pool(name="mpool", bufs=2, space="PSUM") as mpool:
            for ci, (d0, dn) in enumerate(c.dch):
                pst = stpool.tile([128, c.BC], BF16, tag="pst")
                nc.tensor.transpose(
                    out=pst[:],
                    in_=sattn_sb[:, d0:d0 + 128],
                    identity=wb16_sb[0:c.BC, c.IDB:c.IDB + c.BC])
                nc.any.tensor_copy(out=satd[:, ci, :], in_=pst[:])
            for hi in range(c.nh):
                pm = mpool.tile([c.HCW, c.BC], F32, tag="pm")
                k = 0
                for blk in range(3):
                    for ci in range(c.nd):
                        rhs = satd[:, ci, :] if blk == 0 else pooled[:, blk - 1, ci, :]
                        nc.tensor.matmul(
                            out=pm[:],
                            lhsT=w16(c.MW1 + (blk * 3 + ci) * 200 + hi * 100, 100),
                            rhs=rhs, start=(k == 0), stop=(k == 8))
                        k += 1
                nc.scalar.activation(
                    out=o1_sb[:, hi, :], in_=pm[:], func=AF.Relu,
                    bias=wb32_sb[0:c.HCW, 2 + hi:3 + hi], scale=1.0)
            for hi in range(c.nh):
                pm2 = mpool.tile([c.HCW, c.BC], F32, tag="pm2")
                for ki in range(c.nh):
                    nc.tensor.matmul(
                        out=pm2[:],
                        lhsT=wb16_sb[0:c.HCW,
                                     c.MW2 + (ki * 2 + hi) * 100:
                                     c.MW2 + (ki * 2 + hi) * 100 + 100],
                        rhs=o1_sb[:, ki, :],
                        start=(ki == 0), stop=(ki == c.nh - 1))
                nc.scalar.activation(
                    out=out_sb[:, hi, :], in_=pm2[:], func=AF.Relu,
                    bias=wb32_sb[0:c.HCW, 4 + hi:5 + hi], scale=1.0)
            for hi in range(c.nh):
                nc.sync.dma_start(out=out_d[hi], in_=out_sb[:, hi, :])

    nc.finalize()
    return nc


# ------------------------------------------------------------------ host


def wrap16(idx, n):
    """int16 index list -> [128, n/16] wrapped + replicated per Q7 core."""
    return np.ascontiguousarray(
        np.tile(idx.astype(np.int16).reshape(n // 16, 16).T, (8, 1)))


def pad_slots(vals, cap):
    """Pad participant index list to cap slots by repeating the first."""
    assert 1 <= len(vals) <= cap, len(vals)
    out = np.empty(cap, np.int64)
    out[:len(vals)] = vals
    out[len(vals):] = vals[0]
    return out


def host_prepare(cfg: Cfg, words, subj_pos, obj_pos, emb_table,
                 w1, b1, w2, b2, mw1, mb1, mw2, mb2):
    c = cfg
    words = np.asarray(words).astype(np.int64)
    subj_pos = np.asarray(subj_pos)
    obj_pos = np.asarray(obj_pos)
    f32 = lambda x: np.asarray(x, dtype=np.float32)

    table = np.zeros((c.V, c.E), BFNP)
    table[:, :c.D] = f32(emb_table).astype(BFNP)

    w1 = f32(w1)
    mw1 = f32(mw1)
    mw1e = np.concatenate([mw1[0:c.D] + mw1[c.D:2 * c.D],
                           mw1[2 * c.D:3 * c.D], mw1[3 * c.D:4 * c.D]], axis=0)
    mw2 = f32(mw2)
    w2v = f32(w2).reshape(c.H)

    def padD(m):  # [300, H] -> [384, H]
        out = np.zeros((c.DP, m.shape[1]), np.float32)
        out[:c.D] = m
        return out

    w1a = padD(w1[:c.D])
    w1b = padD(w1[c.D:2 * c.D])
    mw1p = np.concatenate([padD(mw1e[i * c.D:(i + 1) * c.D]) for i in range(3)],
                          axis=0)  # [1152, 200]

    wb16 = np.zeros((128, c.NC16), np.float32)
    for ci in range(3):
        wb16[:, c.W1A + ci * 200:c.W1A + (ci + 1) * 200] = w1a[ci * 128:(ci + 1) * 128]
        wb16[:, c.W1B + ci * 200:c.W1B + (ci + 1) * 200] = w1b[ci * 128:(ci + 1) * 128]
    for kk in range(9):
        wb16[:, c.MW1 + kk * 200:c.MW1 + (kk + 1) * 200] = \
            mw1p[kk * 128:(kk + 1) * 128]
    for ki in range(2):
        for hi in range(2):
            wb16[:c.HCW, c.MW2 + (ki * 2 + hi) * 100:
                 c.MW2 + (ki * 2 + hi) * 100 + 100] = \
                mw2[ki * 100:(ki + 1) * 100, hi * 100:(hi + 1) * 100]
    for hi in range(2):
        wb16[:c.HCW, c.W2 + hi] = w2v[hi * 100:(hi + 1) * 100]
    wb16[:, c.IDB:c.IDB + 128] = np.eye(128, dtype=np.float32)
    wb16b = np.ascontiguousarray(wb16.astype(BFNP))

    wb32 = np.zeros((128, 6), np.float32)
    wb32[:c.HCW, 0] = f32(b1)[:c.HCW]
    wb32[:c.HCW, 1] = f32(b1)[c.HCW:]
    wb32[:c.HCW, 2] = f32(mb1)[:c.HCW]
    wb32[:c.HCW, 3] = f32(mb1)[c.HCW:]
    wb32[:c.HCW, 4] = f32(mb2)[:c.HCW]
    wb32[:c.HCW, 5] = f32(mb2)[c.HCW:]

    in_maps = []
    for core in range(c.NCORES):
        b0 = core * c.BC
        # per-batch sorted token lists + per-window participant lists
        lo_tok, hi_tok = [], []          # [BC][256]
        part = {}                        # (b, m, w) -> 64 padded idx
        for b in range(c.BC):
            w = words[b0 + b]
            order = np.argsort(w, kind="stable")
            ws = w[order]
            if ws[c.HALF - 1] >= c.LO_MAX or ws[c.HALF] < c.HB:
                raise RuntimeError(
                    f"batch {b0 + b}: vocab split infeasible "
                    f"(lo_max={ws[c.HALF - 1]}, hi_min={ws[c.HALF]})")
            lo_tok.append(ws[:c.HALF])
            hi_tok.append(ws[c.HALF:] - c.HB)
            for m, pos in ((0, subj_pos), (1, obj_pos)):
                pm = (np.asarray(pos[b0 + b])[order] == 0)   # participates
                part[(b, m, 0)] = pad_slots(ws[:c.HALF][pm[:c.HALF]], c.PCAP)
                part[(b, m, 1)] = pad_slots(
                    ws[c.HALF:][pm[c.HALF:]] - c.HB, c.PCAP)
        # build per-split gather index lists
        lo_list, hi_list = [], []
        for k in range(c.nsp):
            for b in range(4 * k, 4 * k + 4):
                lo_list.append(lo_tok[b])
            for p in (2 * k, 2 * k + 1):
                for m in range(2):
                    lo_list.append(part[(2 * p, m, 0)])
                    lo_list.append(part[(2 * p + 1, m, 0)])
            for b in range(4 * k, 4 * k + 4):
                hi_list.append(hi_tok[b])
            for p in (2 * k, 2 * k + 1):
                for m in range(2):
                    hi_list.append(part[(2 * p, m, 1)])
                    hi_list.append(part[(2 * p + 1, m, 1)])
        nrows = c.NPC * c.nsp
        idx = np.concatenate([wrap16(np.concatenate(lo_list), nrows),
                              wrap16(np.concatenate(hi_list), nrows)], axis=1)
        in_maps.append({
            "idx": np.ascontiguousarray(idx),
            "wb16": wb16b,
            "wb32": wb32,
            "table": table,
        })
    return in_maps


def assemble_output(cfg: Cfg, results):
    c = cfg
    outs = []
    for core in range(c.NCORES):
        o = results[core]["out"]                      # [nh, HCW, BC]
        outs.append(np.transpose(o, (2, 0, 1)).reshape(c.BC, c.H))
    return np.ascontiguousarray(np.concatenate(outs, axis=0))


_CACHE = {}


def run(inputs, trace=False, **kw):
    from concourse.bass_utils import run_bass_kernel_spmd

    cfg = Cfg()
    in_maps = host_prepare(cfg, **{k: inputs[k] for k in (
        "words", "subj_pos", "obj_pos", "emb_table", "w1", "b1", "w2", "b2",
        "mw1", "mb1", "mw2", "mb2")})
    if "nc" not in _CACHE:
        _CACHE["nc"] = build_nc(cfg)
    nc = _CACHE["nc"]
    res = run_bass_kernel_spmd(nc, in_maps, core_ids=list(range(cfg.NCORES)),
                               trace=trace, **kw)
    return assemble_output(cfg, res.results), res


def kernel(**inputs) -> np.ndarray:
    return run(inputs)[0]


# revision 12
# speedup vs baseline: 1.6206x; 1.1051x over previous
"""Trainium2 Bass kernel for nn_AttentionNet (pooling / ridge regime).

Model (per batch b of B=128, L=512, D=300, H=200, V=50000):
  word_emb = emb_table[words]                          [B,L,D]
  subj_emb = max over l with subj_pos[b,l]==0 of word_emb (else -1e12)
  obj_emb  = same with obj_pos
  hid  = tanh(word_emb @ w1[:D] + subj_emb @ w1[D:] + b1)
  attn = softmax(hid @ w2, axis=l)    (b2 dropped: softmax shift-invariant)
  subj_attn = sum_l attn * word_emb   (obj_attn identical -- source bug)
  out = relu(relu(cat([subj_attn, subj_attn, subj_emb, obj_emb]) @ mw1 + mb1) @ mw2 + mb2)

Sharding: pure data parallel, 16 batches per core on 8 cores; embedding
table (bf16) and the small weights replicated.

Device plan per core (16 batches, bf16 everywhere, fp32 PSUM/biases):
  - table rows padded to 384 bf16 cols (768 B = the 256B gather
    granularity); pad values 0.0 meet zero weight rows downstream.
  - the masked max-pools are precomputed as GATHERED PARTICIPANT
    SUBTILES: the host collects, per (batch, mask, vocab-window), the
    <=64 tokens that participate in the pool (pos==0) and pads them to
    64 slots by repeating the first participant (max unchanged).  A
    batch PAIR's two 64-slot sets form one 128-row subtile.  The pool is
    then 3 PE transposes + a half-range PSUM max-reduce per subtile --
    no mask tensors, no mask arithmetic anywhere.
  - scores exp is fused into the PSUM->SBUF score copy (softmax without
    max-shift: |score| <= ~10 so exp is safe in bf16/f32), normalization
    happens batch-major after one small DMA.
  - gathers are issued first (shared num_idxs register, identity shipped
    inside the weight blob so gpsimd never switches ucode libraries for
    compute) and the per-group compute chases gather pairs.
  - per group of 4 batches: PE-transpose embeddings to D-major, dense
    hid/score matmuls, per-group softmax + attention-weighted sum so the
    attention tail pipelines with the next group's matmul stream.
"""

import numpy as np
import ml_dtypes

import concourse.bass as bass
import concourse.bacc as bacc
import concourse.mybir as mybir
import concourse.tile as tile
from contextlib import ExitStack

F32 = mybir.dt.float32
BF16 = mybir.dt.bfloat16
I16 = mybir.dt.int16

NEG_INF = 1e12      # reference constant
BFNP = ml_dtypes.bfloat16

AX = mybir.AxisListType.X
MAX = mybir.AluOpType.max
ADD = mybir.AluOpType.add
AF = mybir.ActivationFunctionType


class Cfg:
    def __init__(self):
        self.B, self.L, self.D, self.H, self.V = 128, 512, 300, 200, 50000
        self.NCORES = 8
        self.BC = 16                  # batches per core
        self.NSUB = 4                 # token subtiles per batch
        self.DP = 384                 # padded D (3 chunks of 128)
        self.E = 384                  # gather row elems (bf16) = 768 B
        self.HCW = 100                # H chunk width
        self.nh = 2
        self.nd = 3
        self.GB = 4                   # batches per group
        self.NG = 4
        self.HALF = 256               # tokens per vocab half
        self.LO_MAX = 32768
        self.HB = self.V - 32768      # high window base
        self.nsp = 4                  # gather splits per half
        self.PCAP = 64                # participant slots per (batch, window)
        # per half-split: 8 batch subtiles + 4 participant subtiles
        self.SPS = 12
        self.NS = 2 * self.nsp * self.SPS     # 96 subtiles
        self.NPC = 128 * self.SPS             # 1536 rows per gather
        self.dch = [(0, 128), (128, 128), (256, 128)]
        # wb16 blob column offsets
        self.W1A = 0
        self.W1B = 600
        self.MW1 = 1200
        self.MW2 = 3000
        self.W2 = 3400
        self.IDB = 3402
        self.NC16 = self.IDB + 128

    def sb_lo(self, b):
        return 12 * (b // 4) + 2 * (b % 4)

    def subtiles(self, b):
        lo = self.sb_lo(b)
        return [lo, lo + 1, 48 + lo, 48 + lo + 1]

    def pp(self, p, m, w):
        """Participant subtile for batch pair p, mask m, window w."""
        return (0 if w == 0 else 48) + 12 * (p // 2) + 8 + 2 * (p % 2) + m


def build_nc(cfg: Cfg):
    c = cfg
    nc = bacc.Bacc(num_swdge_queues=4)

    nix = c.NPC * c.nsp // 16                 # idx cols per half
    idx_d = nc.declare_dram_parameter("idx", [128, 2 * nix], I16, isOutput=False)
    wb16_d = nc.declare_dram_parameter("wb16", [128, c.NC16], BF16, isOutput=False)
    wb32_d = nc.declare_dram_parameter("wb32", [128, 6], F32, isOutput=False)
    table = nc.declare_dram_parameter("table", [c.V, c.E], BF16, isOutput=False)
    out_d = nc.declare_dram_parameter("out", [c.nh, c.HCW, c.BC], F32, isOutput=True)

    with tile.TileContext(nc) as tc, ExitStack() as ctx:
        sb = ctx.enter_context(tc.tile_pool(name="sb", bufs=1))

        ix_sb = sb.tile([128, 2 * nix], I16)
        wb16_sb = sb.tile([128, c.NC16], BF16)
        wb32_sb = sb.tile([128, 6], F32)
        emb_tok = sb.tile([128, c.NS, c.E], BF16)
        pooledf = sb.tile([128, 2, c.nd, c.BC], BF16)
        pooled2 = sb.tile([128, 2, c.nd, c.BC], BF16)
        pooled = sb.tile([128, 2, c.nd, c.BC], BF16)
        bias_sb = sb.tile([c.HCW, c.nh, c.BC], F32)
        attn_t = sb.tile([128, c.NSUB, c.BC], BF16)
        sattn_sb = sb.tile([c.BC, c.E], BF16)
        satd = sb.tile([128, c.nd, c.BC], BF16)
        o1_sb = sb.tile([c.HCW, c.nh, c.BC], BF16)
        out_sb = sb.tile([c.HCW, c.nh, c.BC], F32)

        identb = wb16_sb[:, c.IDB:c.IDB + 128]

        # index DMA first: it gates gather descriptor generation
        nc.sync.dma_start(out=ix_sb[:], in_=idx_d[:])
        nc.scalar.dma_start(out=wb16_sb[:], in_=wb16_d[:])
        nc.scalar.dma_start(out=wb32_sb[:], in_=wb32_d[:])

        # gathers: lo/hi pair per split k; pair k feeds batch group k
        npc = c.NPC
        nreg = nc.gpsimd.to_reg(npc)
        for k in range(c.nsp):
            i0 = k * (npc // 16)
            nc.gpsimd.dma_gather(
                out_ap=emb_tok[:, k * c.SPS:(k + 1) * c.SPS, :],
                in_ap=table[0:c.LO_MAX, :],
                idxs_ap=ix_sb[:, i0:i0 + npc // 16],
                num_idxs=npc, num_idxs_reg=nreg, elem_size=c.E,
                single_packet=False, queue_num=(2 * k) % 4)
            nc.gpsimd.dma_gather(
                out_ap=emb_tok[:, 48 + k * c.SPS:48 + (k + 1) * c.SPS, :],
                in_ap=table[c.HB:c.V, :],
                idxs_ap=ix_sb[:, nix + i0:nix + i0 + npc // 16],
                num_idxs=npc, num_idxs_reg=nreg, elem_size=c.E,
                single_packet=False, queue_num=(2 * k + 1) % 4)

        def w16(col, n, p=128):
            return wb16_sb[0:p, col:col + n]

        with tc.tile_pool(name="edpool", bufs=8) as edpool, \
             tc.tile_pool(name="hidpool", bufs=2) as hidpool, \
             tc.tile_pool(name="sstpool", bufs=2) as sstpool, \
             tc.tile_pool(name="saspool", bufs=2) as saspool, \
             tc.tile_pool(name="scgpool", bufs=2) as scgpool, \
             tc.tile_pool(name="smpool", bufs=2) as smpool, \
             tc.tile_pool(name="tepool", bufs=2, space="PSUM") as tepool, \
             tc.tile_pool(name="tppool", bufs=2, space="PSUM") as tppool, \
             tc.tile_pool(name="hpool", bufs=2, space="PSUM") as hpool, \
             tc.tile_pool(name="spool", bufs=1, space="PSUM") as spool, \
             tc.tile_pool(name="wpool", bufs=1, space="PSUM") as wpool:
            ssts = {}

            def attn_phase(ag):
                a0 = ag * c.GB
                asl = slice(a0, a0 + c.GB)
                scg = scgpool.tile([c.GB, c.L], BF16, tag="sc")
                nc.sync.dma_start(out=scg[:], in_=ssts[ag][:])
                sm = smpool.tile([c.GB, 2], F32, tag="sm")
                nc.vector.tensor_reduce(out=sm[:, 0:1], in_=scg[:], axis=AX,
                                        op=ADD)
                nc.vector.reciprocal(out=sm[:, 1:2], in_=sm[:, 0:1])
                atb = scgpool.tile([c.GB, c.L], BF16, tag="atb")
                nc.any.tensor_scalar_mul(out=atb[:], in0=scg[:],
                                         scalar1=sm[:, 1:2])
                for si in range(c.NSUB):
                    tp = tppool.tile([128, c.nd, 128], BF16, tag="tp")
                    nc.tensor.transpose(
                        out=tp[:, 0, 0:c.GB],
                        in_=atb[:, si * 128:(si + 1) * 128],
                        identity=wb16_sb[0:c.GB, c.IDB:c.IDB + c.GB])
                    nc.any.tensor_copy(out=attn_t[:, si, asl],
                                       in_=tp[:, 0, 0:c.GB])
                sas = saspool.tile([1, c.GB, c.E], BF16, tag="sas")
                for bi, b in enumerate(range(a0, a0 + c.GB)):
                    pw = wpool.tile([1, c.E], F32, tag="pw")
                    for si, s in enumerate(c.subtiles(b)):
                        nc.tensor.matmul(
                            out=pw[:],
                            lhsT=attn_t[:, si, b:b + 1],
                            rhs=emb_tok[:, s, :],
                            start=(si == 0), stop=(si == c.NSUB - 1))
                    if bi % 2 == 0:
                        nc.vector.tensor_copy(out=sas[0:1, bi, :], in_=pw[:])
                    else:
                        nc.scalar.copy(out=sas[0:1, bi, :], in_=pw[:])
                nc.scalar.dma_start(out=sattn_sb[asl, :], in_=sas[:])

            for g in range(c.NG):
                g0 = g * c.GB
                gsl = slice(g0, g0 + c.GB)
                eds = []
                for b in range(g0, g0 + c.GB):
                    subs = c.subtiles(b)
                    ed = edpool.tile([128, c.nd, c.L], BF16, tag="ed")
                    for ci, (d0, dn) in enumerate(c.dch):
                        te = tepool.tile([128, c.NSUB, 128], BF16, tag="te")
                        for si, s in enumerate(subs):
                            nc.tensor.transpose(
                                out=te[:, si, :],
                                in_=emb_tok[:, s, d0:d0 + 128],
                                identity=identb)
                        if ci == 0:
                            nc.scalar.copy(out=ed[:, ci, :], in_=te[:])
                        else:
                            nc.vector.tensor_copy(out=ed[:, ci, :], in_=te[:])
                    eds.append(ed)
                # pools from participant subtiles (batch pairs 2g, 2g+1)
                for pi in range(2):
                    p = 2 * g + pi
                    for m in range(2):
                        for w, dst in ((0, pooledf), (1, pooled2)):
                            tp = tppool.tile([128, c.nd, 128], BF16, tag="tp")
                            s = c.pp(p, m, w)
                            for ci, (d0, dn) in enumerate(c.dch):
                                nc.tensor.transpose(
                                    out=tp[:, ci, :],
                                    in_=emb_tok[:, s, d0:d0 + 128],
                                    identity=identb)
                            nc.vector.tensor_reduce(
                                out=dst[:, m, :, 2 * p], in_=tp[:, :, 0:c.PCAP],
                                axis=AX, op=MAX)
                            nc.vector.tensor_reduce(
                                out=dst[:, m, :, 2 * p + 1],
                                in_=tp[:, :, c.PCAP:128],
                                axis=AX, op=MAX)
                # combine windows (clamp unnecessary: participants are real)
                nc.any.tensor_tensor(
                    out=pooled[:, :, :, gsl], in0=pooledf[:, :, :, gsl],
                    in1=pooled2[:, :, :, gsl], op=MAX)
                # tanh bias for the group: w1b^T subj_emb + b1
                for hi in range(c.nh):
                    pb = hpool.tile([c.HCW, c.L], F32, tag="ph")
                    for ci in range(c.nd):
                        nc.tensor.matmul(
                            out=pb[:, 0:c.GB],
                            lhsT=w16(c.W1B + ci * 200 + hi * 100, 100),
                            rhs=pooled[:, 0, ci, gsl],
                            start=(ci == 0), stop=(ci == c.nd - 1))
                    nc.scalar.activation(
                        out=bias_sb[:, hi, gsl], in_=pb[:, 0:c.GB],
                        func=AF.Identity, bias=wb32_sb[0:c.HCW, hi:hi + 1],
                        scale=1.0)
                if g > 0:
                    attn_phase(g - 1)
                # dense hid + scores per batch; exp fused into score copy
                sst = sstpool.tile([1, c.GB, c.L], BF16, tag="sst")
                ssts[g] = sst
                for bi, b in enumerate(range(g0, g0 + c.GB)):
                    ed = eds[bi]
                    hid = hidpool.tile([c.HCW, c.nh, c.L], BF16, tag="hid")
                    for hi in range(c.nh):
                        ph = hpool.tile([c.HCW, c.L], F32, tag="ph")
                        for ci in range(c.nd):
                            nc.tensor.matmul(
                                out=ph[:],
                                lhsT=w16(c.W1A + ci * 200 + hi * 100, 100),
                                rhs=ed[:, ci, :],
                                start=(ci == 0), stop=(ci == c.nd - 1))
                        nc.scalar.activation(
                            out=hid[:, hi, :], in_=ph[:],
                            func=AF.Tanh, bias=bias_sb[:, hi, b:b + 1],
                            scale=1.0)
                    ps = spool.tile([1, c.L], F32, tag="ps")
                    for hi in range(c.nh):
                        nc.tensor.matmul(
                            out=ps[:],
                            lhsT=wb16_sb[0:c.HCW, c.W2 + hi:c.W2 + hi + 1],
                            rhs=hid[:, hi, :],
                            start=(hi == 0), stop=(hi == c.nh - 1))
                    nc.scalar.activation(out=sst[0:1, bi, :], in_=ps[:],
                                         func=AF.Exp)

            attn_phase(c.NG - 1)

        # ---- tail: satd transposes + output MLP
        with tc.tile_pool(name="stpool", bufs=1, space="PSUM") as stpool, \
             tc.tile_pool(name="mpool", bufs=2, space="PSUM") as mpool:
            for ci, (d0, dn) in enumerate(c.dch):
                pst = stpool.tile([128, c.BC], BF16, tag="pst")
                nc.tensor.transpose(
                    out=pst[:],
                    in_=sattn_sb[:, d0:d0 + 128],
                    identity=wb16_sb[0:c.BC, c.IDB:c.IDB + c.BC])
                nc.any.tensor_copy(out=satd[:, ci, :], in_=pst[:])
            for hi in range(c.nh):
                pm = mpool.tile([c.HCW, c.BC], F32, tag="pm")
                k = 0
                for blk in range(3):
                    for ci in range(c.nd):
                        rhs = satd[:, ci, :] if blk == 0 else pooled[:, blk - 1, ci, :]
                        nc.tensor.matmul(
                            out=pm[:],
                            lhsT=w16(c.MW1 + (blk * 3 + ci) * 200 + hi * 100, 100),
                            rhs=rhs, start=(k == 0), stop=(k == 8))
                        k += 1
                nc.scalar.activation(
                    out=o1_sb[:, hi, :], in_=pm[:], func=AF.Relu,
                    bias=wb32_sb[0:c.HCW, 2 + hi:3 + hi], scale=1.0)
            for hi in range(c.nh):
                pm2 = mpool.tile([c.HCW, c.BC], F32, tag="pm2")
                for ki in range(c.nh):
                    nc.tensor.matmul(
                        out=pm2[:],
                        lhsT=wb16_sb[0:c.HCW,
                                     c.MW2 + (ki * 2 + hi) * 100:
                                     c.MW2 + (ki * 2 + hi) * 100 + 100],
                        rhs=o1_sb[:, ki, :],
                        start=(ki == 0), stop=(ki == c.nh - 1))
                nc.scalar.activation(
                    out=out_sb[:, hi, :], in_=pm2[:], func=AF.Relu,
                    bias=wb32_sb[0:c.HCW, 4 + hi:5 + hi], scale=1.0)
            for hi in range(c.nh):
                nc.sync.dma_start(out=out_d[hi], in_=out_sb[:, hi, :])

    nc.finalize()
    return nc


# ------------------------------------------------------------------ host


def wrap16(idx, n):
    """int16 index list -> [128, n/16] wrapped + replicated per Q7 core."""
    return np.ascontiguousarray(
        np.tile(idx.astype(np.int16).reshape(n // 16, 16).T, (8, 1)))


def pad_slots(vals, cap):
    """Pad participant index list to cap slots by repeating the first."""
    assert 1 <= len(vals) <= cap, len(vals)
    out = np.empty(cap, np.int64)
    out[:len(vals)] = vals
    out[len(vals):] = vals[0]
    return out


def host_prepare(cfg: Cfg, words, subj_pos, obj_pos, emb_table,
                 w1, b1, w2, b2, mw1, mb1, mw2, mb2):
    c = cfg
    words = np.asarray(words).astype(np.int64)
    subj_pos = np.asarray(subj_pos)
    obj_pos = np.asarray(obj_pos)
    f32 = lambda x: np.asarray(x, dtype=np.float32)

    table = np.zeros((c.V, c.E), BFNP)
    table[:, :c.D] = f32(emb_table).astype(BFNP)

    w1 = f32(w1)
    mw1 = f32(mw1)
    mw1e = np.concatenate([mw1[0:c.D] + mw1[c.D:2 * c.D],
                           mw1[2 * c.D:3 * c.D], mw1[3 * c.D:4 * c.D]], axis=0)
    mw2 = f32(mw2)
    w2v = f32(w2).reshape(c.H)

    def padD(m):  # [300, H] -> [384, H]
        out = np.zeros((c.DP, m.shape[1]), np.float32)
        out[:c.D] = m
        return out

    w1a = padD(w1[:c.D])
    w1b = padD(w1[c.D:2 * c.D])
    mw1p = np.concatenate([padD(mw1e[i * c.D:(i + 1) * c.D]) for i in range(3)],
                          axis=0)  # [1152, 200]

    wb16 = np.zeros((128, c.NC16), np.float32)
    for ci in range(3):
        wb16[:, c.W1A + ci * 200:c.W1A + (ci + 1) * 200] = w1a[ci * 128:(ci + 1) * 128]
        wb16[:, c.W1B + ci * 200:c.W1B + (ci + 1) * 200] = w1b[ci * 128:(ci + 1) * 128]
    for kk in range(9):
        wb16[:, c.MW1 + kk * 200:c.MW1 + (kk + 1) * 200] = \
            mw1p[kk * 128:(kk + 1) * 128]
    for ki in range(2):
        for hi in range(2):
            wb16[:c.HCW, c.MW2 + (ki * 2 + hi) * 100:
                 c.MW2 + (ki * 2 + hi) * 100 + 100] = \
                mw2[ki * 100:(ki + 1) * 100, hi * 100:(hi + 1) * 100]
    for hi in range(2):
        wb16[:c.HCW, c.W2 + hi] = w2v[hi * 100:(hi + 1) * 100]
    wb16[:, c.IDB:c.IDB + 128] = np.eye(128, dtype=np.float32)
    wb16b = np.ascontiguousarray(wb16.astype(BFNP))

    wb32 = np.zeros((128, 6), np.float32)
    wb32[:c.HCW, 0] = f32(b1)[:c.HCW]
    wb32[:c.HCW, 1] = f32(b1)[c.HCW:]
    wb32[:c.HCW, 2] = f32(mb1)[:c.HCW]
    wb32[:c.HCW, 3] = f32(mb1)[c.HCW:]
    wb32[:c.HCW, 4] = f32(mb2)[:c.HCW]
    wb32[:c.HCW, 5] = f32(mb2)[c.HCW:]

    in_maps = []
    for core in range(c.NCORES):
        b0 = core * c.BC
        # per-batch sorted token lists + per-window participant lists
        lo_tok, hi_tok = [], []          # [BC][256]
        part = {}                        # (b, m, w) -> 64 padded idx
        for b in range(c.BC):
            w = words[b0 + b]
            order = np.argsort(w, kind="stable")
            ws = w[order]
            if ws[c.HALF - 1] >= c.LO_MAX or ws[c.HALF] < c.HB:
                raise RuntimeError(
                    f"batch {b0 + b}: vocab split infeasible "
                    f"(lo_max={ws[c.HALF - 1]}, hi_min={ws[c.HALF]})")
            lo_tok.append(ws[:c.HALF])
            hi_tok.append(ws[c.HALF:] - c.HB)
            for m, pos in ((0, subj_pos), (1, obj_pos)):
                pm = (np.asarray(pos[b0 + b])[order] == 0)   # participates
                part[(b, m, 0)] = pad_slots(ws[:c.HALF][pm[:c.HALF]], c.PCAP)
                part[(b, m, 1)] = pad_slots(
                    ws[c.HALF:][pm[c.HALF:]] - c.HB, c.PCAP)
        # build per-split gather index lists
        lo_list, hi_list = [], []
        for k in range(c.nsp):
            for b in range(4 * k, 4 * k + 4):
                lo_list.append(lo_tok[b])
            for p in (2 * k, 2 * k + 1):
                for m in range(2):
                    lo_list.append(part[(2 * p, m, 0)])
                    lo_list.append(part[(2 * p + 1, m, 0)])
            for b in range(4 * k, 4 * k + 4):
                hi_list.append(hi_tok[b])
            for p in (2 * k, 2 * k + 1):
                for m in range(2):
                    hi_list.append(part[(2 * p, m, 1)])
                    hi_list.append(part[(2 * p + 1, m, 1)])
        nrows = c.NPC * c.nsp
        idx = np.concatenate([wrap16(np.concatenate(lo_list), nrows),
                              wrap16(np.concatenate(hi_list), nrows)], axis=1)
        in_maps.append({
            "idx": np.ascontiguousarray(idx),
            "wb16": wb16b,
            "wb32": wb32,
            "table": table,
        })
    return in_maps


def assemble_output(cfg: Cfg, results):
    c = cfg
    outs = []
    for core in range(c.NCORES):
        o = results[core]["out"]                      # [nh, HCW, BC]
        outs.append(np.transpose(o, (2, 0, 1)).reshape(c.BC, c.H))
    return np.ascontiguousarray(np.concatenate(outs, axis=0))


_CACHE = {}


def run(inputs, trace=False, **kw):
    from concourse.bass_utils import run_bass_kernel_spmd

    cfg = Cfg()
    in_maps = host_prepare(cfg, **{k: inputs[k] for k in (
        "words", "subj_pos", "obj_pos", "emb_table", "w1", "b1", "w2", "b2",
        "mw1", "mb1", "mw2", "mb2")})
    if "nc" not in _CACHE:
        _CACHE["nc"] = build_nc(cfg)
    nc = _CACHE["nc"]
    res = run_bass_kernel_spmd(nc, in_maps, core_ids=list(range(cfg.NCORES)),
                               trace=trace, **kw)
    return assemble_output(cfg, res.results), res


def kernel(**inputs) -> np.ndarray:
    return run(inputs)[0]


# revision 13
# speedup vs baseline: 1.7593x; 1.0855x over previous
"""Trainium2 Bass kernel for nn_AttentionNet (pooling / ridge regime).

Model (per batch b of B=128, L=512, D=300, H=200, V=50000):
  word_emb = emb_table[words]                          [B,L,D]
  subj_emb = max over l with subj_pos[b,l]==0 of word_emb (else -1e12)
  obj_emb  = same with obj_pos
  hid  = tanh(word_emb @ w1[:D] + subj_emb @ w1[D:] + b1)
  attn = softmax(hid @ w2, axis=l)    (b2 dropped: softmax shift-invariant)
  subj_attn = sum_l attn * word_emb   (obj_attn identical -- source bug)
  out = relu(relu(cat([subj_attn, subj_attn, subj_emb, obj_emb]) @ mw1 + mb1) @ mw2 + mb2)

Sharding: pure data parallel, 16 batches per core on 8 cores; embedding
table (bf16) and the small weights replicated.

Device plan per core (16 batches, bf16 everywhere, fp32 PSUM/biases):
  - table rows padded to 384 bf16 cols (768 B = the 256B gather
    granularity); pad values 0.0 meet zero weight rows downstream.
  - the masked max-pools are precomputed as GATHERED PARTICIPANT
    SUBTILES: the host collects, per (batch, mask, vocab-window), the
    <=64 tokens that participate in the pool (pos==0) and pads them to
    64 slots by repeating the first participant (max unchanged).  A
    batch PAIR's two 64-slot sets form one 128-row subtile.  The pool is
    then 3 PE transposes + a half-range PSUM max-reduce per subtile --
    no mask tensors, no mask arithmetic anywhere.
  - scores exp is fused into the PSUM->SBUF score copy (softmax without
    max-shift: |score| <= ~10 so exp is safe in bf16/f32), normalization
    happens batch-major after one small DMA.
  - gathers are issued first (shared num_idxs register, identity shipped
    inside the weight blob so gpsimd never switches ucode libraries for
    compute) and the per-group compute chases gather pairs.
  - per group of 4 batches: PE-transpose embeddings to D-major, dense
    hid/score matmuls, per-group softmax + attention-weighted sum so the
    attention tail pipelines with the next group's matmul stream.
"""

import numpy as np
import ml_dtypes

import concourse.bass as bass
import concourse.bacc as bacc
import concourse.mybir as mybir
import concourse.tile as tile
from contextlib import ExitStack

F32 = mybir.dt.float32
BF16 = mybir.dt.bfloat16
I16 = mybir.dt.int16

NEG_INF = 1e12      # reference constant
BFNP = ml_dtypes.bfloat16

AX = mybir.AxisListType.X
MAX = mybir.AluOpType.max
ADD = mybir.AluOpType.add
AF = mybir.ActivationFunctionType


class Cfg:
    def __init__(self):
        self.B, self.L, self.D, self.H, self.V = 128, 512, 300, 200, 50000
        self.NCORES = 8
        self.BC = 16                  # batches per core
        self.NSUB = 4                 # token subtiles per batch
        self.DP = 384                 # padded D (3 chunks of 128)
        self.E = 384                  # gather row elems (bf16) = 768 B
        self.HCW = 100                # H chunk width
        self.nh = 2
        self.nd = 3
        self.GB = 4                   # batches per group
        self.NG = 4
        self.HALF = 256               # tokens per vocab half
        self.LO_MAX = 32768
        self.HB = self.V - 32768      # high window base
        self.nsp = 8                  # gather splits per half
        self.PCAP = 64                # participant slots per (batch, window)
        # per half-split: 4 batch subtiles + 2 participant subtiles
        self.SPS = 6
        self.NS = 2 * self.nsp * self.SPS     # 96 subtiles
        self.NPC = 128 * self.SPS             # 1536 rows per gather
        self.dch = [(0, 128), (128, 128), (256, 128)]
        # wb16 blob column offsets
        self.W1A = 0
        self.W1B = 600
        self.MW1 = 1200
        self.MW2 = 3000
        self.W2 = 3400
        self.IDB = 3402
        self.NC16 = self.IDB + 128

    def sb_lo(self, b):
        return 6 * (b // 2) + 2 * (b % 2)

    def subtiles(self, b):
        lo = self.sb_lo(b)
        return [lo, lo + 1, 48 + lo, 48 + lo + 1]

    def pp(self, p, m, w):
        """Participant subtile for batch pair p, mask m, window w."""
        return (0 if w == 0 else 48) + 6 * p + 4 + m


def build_nc(cfg: Cfg):
    c = cfg
    nc = bacc.Bacc(num_swdge_queues=4)

    nix = c.NPC * c.nsp // 16                 # idx cols per half
    idx_d = nc.declare_dram_parameter("idx", [128, 2 * nix], I16, isOutput=False)
    wb16_d = nc.declare_dram_parameter("wb16", [128, c.NC16], BF16, isOutput=False)
    wb32_d = nc.declare_dram_parameter("wb32", [128, 6], F32, isOutput=False)
    table = nc.declare_dram_parameter("table", [c.V, c.E], BF16, isOutput=False)
    out_d = nc.declare_dram_parameter("out", [c.nh, c.HCW, c.BC], F32, isOutput=True)

    with tile.TileContext(nc) as tc, ExitStack() as ctx:
        sb = ctx.enter_context(tc.tile_pool(name="sb", bufs=1))

        ix_sb = sb.tile([128, 2 * nix], I16)
        wb16_sb = sb.tile([128, c.NC16], BF16)
        wb32_sb = sb.tile([128, 6], F32)
        emb_tok = sb.tile([128, c.NS, c.E], BF16)
        pooledf = sb.tile([128, 2, c.nd, c.BC], BF16)
        pooled2 = sb.tile([128, 2, c.nd, c.BC], BF16)
        pooled = sb.tile([128, 2, c.nd, c.BC], BF16)
        bias_sb = sb.tile([c.HCW, c.nh, c.BC], F32)
        attn_t = sb.tile([128, c.NSUB, c.BC], BF16)
        sattn_sb = sb.tile([c.BC, c.E], BF16)
        satd = sb.tile([128, c.nd, c.BC], BF16)
        o1_sb = sb.tile([c.HCW, c.nh, c.BC], BF16)
        out_sb = sb.tile([c.HCW, c.nh, c.BC], F32)

        identb = wb16_sb[:, c.IDB:c.IDB + 128]

        # index DMA first: it gates gather descriptor generation
        nc.sync.dma_start(out=ix_sb[:], in_=idx_d[:])
        nc.scalar.dma_start(out=wb16_sb[:], in_=wb16_d[:])
        nc.scalar.dma_start(out=wb32_sb[:], in_=wb32_d[:])

        # gathers: lo/hi pair per split k; pair k feeds batch group k
        npc = c.NPC
        nreg = nc.gpsimd.to_reg(npc)
        for k in range(c.nsp):
            i0 = k * (npc // 16)
            nc.gpsimd.dma_gather(
                out_ap=emb_tok[:, k * c.SPS:(k + 1) * c.SPS, :],
                in_ap=table[0:c.LO_MAX, :],
                idxs_ap=ix_sb[:, i0:i0 + npc // 16],
                num_idxs=npc, num_idxs_reg=nreg, elem_size=c.E,
                single_packet=False, queue_num=(2 * k) % 4)
            nc.gpsimd.dma_gather(
                out_ap=emb_tok[:, 48 + k * c.SPS:48 + (k + 1) * c.SPS, :],
                in_ap=table[c.HB:c.V, :],
                idxs_ap=ix_sb[:, nix + i0:nix + i0 + npc // 16],
                num_idxs=npc, num_idxs_reg=nreg, elem_size=c.E,
                single_packet=False, queue_num=(2 * k + 1) % 4)

        def w16(col, n, p=128):
            return wb16_sb[0:p, col:col + n]

        with tc.tile_pool(name="edpool", bufs=8) as edpool, \
             tc.tile_pool(name="hidpool", bufs=2) as hidpool, \
             tc.tile_pool(name="sstpool", bufs=2) as sstpool, \
             tc.tile_pool(name="saspool", bufs=2) as saspool, \
             tc.tile_pool(name="scgpool", bufs=2) as scgpool, \
             tc.tile_pool(name="smpool", bufs=2) as smpool, \
             tc.tile_pool(name="tepool", bufs=2, space="PSUM") as tepool, \
             tc.tile_pool(name="tppool", bufs=2, space="PSUM") as tppool, \
             tc.tile_pool(name="hpool", bufs=2, space="PSUM") as hpool, \
             tc.tile_pool(name="spool", bufs=1, space="PSUM") as spool, \
             tc.tile_pool(name="wpool", bufs=1, space="PSUM") as wpool:
            ssts = {}

            def attn_phase(ag):
                a0 = ag * c.GB
                asl = slice(a0, a0 + c.GB)
                scg = scgpool.tile([c.GB, c.L], BF16, tag="sc")
                nc.sync.dma_start(out=scg[:], in_=ssts[ag][:])
                sm = smpool.tile([c.GB, 2], F32, tag="sm")
                nc.vector.tensor_reduce(out=sm[:, 0:1], in_=scg[:], axis=AX,
                                        op=ADD)
                nc.vector.reciprocal(out=sm[:, 1:2], in_=sm[:, 0:1])
                atb = scgpool.tile([c.GB, c.L], BF16, tag="atb")
                nc.any.tensor_scalar_mul(out=atb[:], in0=scg[:],
                                         scalar1=sm[:, 1:2])
                for si in range(c.NSUB):
                    tp = tppool.tile([128, c.nd, 128], BF16, tag="tp")
                    nc.tensor.transpose(
                        out=tp[:, 0, 0:c.GB],
                        in_=atb[:, si * 128:(si + 1) * 128],
                        identity=wb16_sb[0:c.GB, c.IDB:c.IDB + c.GB])
                    nc.any.tensor_copy(out=attn_t[:, si, asl],
                                       in_=tp[:, 0, 0:c.GB])
                sas = saspool.tile([1, c.GB, c.E], BF16, tag="sas")
                for bi, b in enumerate(range(a0, a0 + c.GB)):
                    pw = wpool.tile([1, c.E], F32, tag="pw")
                    for si, s in enumerate(c.subtiles(b)):
                        nc.tensor.matmul(
                            out=pw[:],
                            lhsT=attn_t[:, si, b:b + 1],
                            rhs=emb_tok[:, s, :],
                            start=(si == 0), stop=(si == c.NSUB - 1))
                    if bi % 2 == 0:
                        nc.vector.tensor_copy(out=sas[0:1, bi, :], in_=pw[:])
                    else:
                        nc.scalar.copy(out=sas[0:1, bi, :], in_=pw[:])
                nc.scalar.dma_start(out=sattn_sb[asl, :], in_=sas[:])

            for g in range(c.NG):
                g0 = g * c.GB
                gsl = slice(g0, g0 + c.GB)
                eds = []
                for b in range(g0, g0 + c.GB):
                    subs = c.subtiles(b)
                    ed = edpool.tile([128, c.nd, c.L], BF16, tag="ed")
                    for ci, (d0, dn) in enumerate(c.dch):
                        te = tepool.tile([128, c.NSUB, 128], BF16, tag="te")
                        for si, s in enumerate(subs):
                            nc.tensor.transpose(
                                out=te[:, si, :],
                                in_=emb_tok[:, s, d0:d0 + 128],
                                identity=identb)
                        if ci == 0:
                            nc.scalar.copy(out=ed[:, ci, :], in_=te[:])
                        else:
                            nc.vector.tensor_copy(out=ed[:, ci, :], in_=te[:])
                    eds.append(ed)
                # pools from participant subtiles (batch pairs 2g, 2g+1)
                for pi in range(2):
                    p = 2 * g + pi
                    for m in range(2):
                        for w, dst in ((0, pooledf), (1, pooled2)):
                            tp = tppool.tile([128, c.nd, 128], BF16, tag="tp")
                            s = c.pp(p, m, w)
                            for ci, (d0, dn) in enumerate(c.dch):
                                nc.tensor.transpose(
                                    out=tp[:, ci, :],
                                    in_=emb_tok[:, s, d0:d0 + 128],
                                    identity=identb)
                            nc.vector.tensor_reduce(
                                out=dst[:, m, :, 2 * p], in_=tp[:, :, 0:c.PCAP],
                                axis=AX, op=MAX)
                            nc.vector.tensor_reduce(
                                out=dst[:, m, :, 2 * p + 1],
                                in_=tp[:, :, c.PCAP:128],
                                axis=AX, op=MAX)
                # combine windows (clamp unnecessary: participants are real)
                nc.any.tensor_tensor(
                    out=pooled[:, :, :, gsl], in0=pooledf[:, :, :, gsl],
                    in1=pooled2[:, :, :, gsl], op=MAX)
                # tanh bias for the group: w1b^T subj_emb + b1
                for hi in range(c.nh):
                    pb = hpool.tile([c.HCW, c.L], F32, tag="ph")
                    for ci in range(c.nd):
                        nc.tensor.matmul(
                            out=pb[:, 0:c.GB],
                            lhsT=w16(c.W1B + ci * 200 + hi * 100, 100),
                            rhs=pooled[:, 0, ci, gsl],
                            start=(ci == 0), stop=(ci == c.nd - 1))
                    nc.scalar.activation(
                        out=bias_sb[:, hi, gsl], in_=pb[:, 0:c.GB],
                        func=AF.Identity, bias=wb32_sb[0:c.HCW, hi:hi + 1],
                        scale=1.0)
                if g > 0:
                    attn_phase(g - 1)
                # dense hid + scores per batch; exp fused into score copy
                sst = sstpool.tile([1, c.GB, c.L], BF16, tag="sst")
                ssts[g] = sst
                for bi, b in enumerate(range(g0, g0 + c.GB)):
                    ed = eds[bi]
                    hid = hidpool.tile([c.HCW, c.nh, c.L], BF16, tag="hid")
                    for hi in range(c.nh):
                        ph = hpool.tile([c.HCW, c.L], F32, tag="ph")
                        for ci in range(c.nd):
                            nc.tensor.matmul(
                                out=ph[:],
                                lhsT=w16(c.W1A + ci * 200 + hi * 100, 100),
                                rhs=ed[:, ci, :],
                                start=(ci == 0), stop=(ci == c.nd - 1))
                        nc.scalar.activation(
                            out=hid[:, hi, :], in_=ph[:],
                            func=AF.Tanh, bias=bias_sb[:, hi, b:b + 1],
                            scale=1.0)
                    ps = spool.tile([1, c.L], F32, tag="ps")
                    for hi in range(c.nh):
                        nc.tensor.matmul(
                            out=ps[:],
                            lhsT=wb16_sb[0:c.HCW, c.W2 + hi:c.W2 + hi + 1],
                            rhs=hid[:, hi, :],
                            start=(hi == 0), stop=(hi == c.nh - 1))
                    nc.scalar.activation(out=sst[0:1, bi, :], in_=ps[:],
                                         func=AF.Exp)

            attn_phase(c.NG - 1)

        # ---- tail: satd transposes + output MLP
        with tc.tile_pool(name="stpool", bufs=1, space="PSUM") as stpool, \
             tc.tile_pool(name="mpool", bufs=2, space="PSUM") as mpool:
            for ci, (d0, dn) in enumerate(c.dch):
                pst = stpool.tile([128, c.BC], BF16, tag="pst")
                nc.tensor.transpose(
                    out=pst[:],
                    in_=sattn_sb[:, d0:d0 + 128],
                    identity=wb16_sb[0:c.BC, c.IDB:c.IDB + c.BC])
                nc.any.tensor_copy(out=satd[:, ci, :], in_=pst[:])
            for hi in range(c.nh):
                pm = mpool.tile([c.HCW, c.BC], F32, tag="pm")
                k = 0
                for blk in range(3):
                    for ci in range(c.nd):
                        rhs = satd[:, ci, :] if blk == 0 else pooled[:, blk - 1, ci, :]
                        nc.tensor.matmul(
                            out=pm[:],
                            lhsT=w16(c.MW1 + (blk * 3 + ci) * 200 + hi * 100, 100),
                            rhs=rhs, start=(k == 0), stop=(k == 8))
                        k += 1
                nc.scalar.activation(
                    out=o1_sb[:, hi, :], in_=pm[:], func=AF.Relu,
                    bias=wb32_sb[0:c.HCW, 2 + hi:3 + hi], scale=1.0)
            for hi in range(c.nh):
                pm2 = mpool.tile([c.HCW, c.BC], F32, tag="pm2")
                for ki in range(c.nh):
                    nc.tensor.matmul(
                        out=pm2[:],
                        lhsT=wb16_sb[0:c.HCW,
                                     c.MW2 + (ki * 2 + hi) * 100:
                                     c.MW2 + (ki * 2 + hi) * 100 + 100],
                        rhs=o1_sb[:, ki, :],
                        start=(ki == 0), stop=(ki == c.nh - 1))
                nc.scalar.activation(
                    out=out_sb[:, hi, :], in_=pm2[:], func=AF.Relu,
                    bias=wb32_sb[0:c.HCW, 4 + hi:5 + hi], scale=1.0)
            for hi in range(c.nh):
                nc.sync.dma_start(out=out_d[hi], in_=out_sb[:, hi, :])

    nc.finalize()
    return nc


# ------------------------------------------------------------------ host


def wrap16(idx, n):
    """int16 index list -> [128, n/16] wrapped + replicated per Q7 core."""
    return np.ascontiguousarray(
        np.tile(idx.astype(np.int16).reshape(n // 16, 16).T, (8, 1)))


def pad_slots(vals, cap):
    """Pad participant index list to cap slots by repeating the first."""
    assert 1 <= len(vals) <= cap, len(vals)
    out = np.empty(cap, np.int64)
    out[:len(vals)] = vals
    out[len(vals):] = vals[0]
    return out


def host_prepare(cfg: Cfg, words, subj_pos, obj_pos, emb_table,
                 w1, b1, w2, b2, mw1, mb1, mw2, mb2):
    c = cfg
    words = np.asarray(words).astype(np.int64)
    subj_pos = np.asarray(subj_pos)
    obj_pos = np.asarray(obj_pos)
    f32 = lambda x: np.asarray(x, dtype=np.float32)

    table = np.zeros((c.V, c.E), BFNP)
    table[:, :c.D] = f32(emb_table).astype(BFNP)

    w1 = f32(w1)
    mw1 = f32(mw1)
    mw1e = np.concatenate([mw1[0:c.D] + mw1[c.D:2 * c.D],
                           mw1[2 * c.D:3 * c.D], mw1[3 * c.D:4 * c.D]], axis=0)
    mw2 = f32(mw2)
    w2v = f32(w2).reshape(c.H)

    def padD(m):  # [300, H] -> [384, H]
        out = np.zeros((c.DP, m.shape[1]), np.float32)
        out[:c.D] = m
        return out

    w1a = padD(w1[:c.D])
    w1b = padD(w1[c.D:2 * c.D])
    mw1p = np.concatenate([padD(mw1e[i * c.D:(i + 1) * c.D]) for i in range(3)],
                          axis=0)  # [1152, 200]

    wb16 = np.zeros((128, c.NC16), np.float32)
    for ci in range(3):
        wb16[:, c.W1A + ci * 200:c.W1A + (ci + 1) * 200] = w1a[ci * 128:(ci + 1) * 128]
        wb16[:, c.W1B + ci * 200:c.W1B + (ci + 1) * 200] = w1b[ci * 128:(ci + 1) * 128]
    for kk in range(9):
        wb16[:, c.MW1 + kk * 200:c.MW1 + (kk + 1) * 200] = \
            mw1p[kk * 128:(kk + 1) * 128]
    for ki in range(2):
        for hi in range(2):
            wb16[:c.HCW, c.MW2 + (ki * 2 + hi) * 100:
                 c.MW2 + (ki * 2 + hi) * 100 + 100] = \
                mw2[ki * 100:(ki + 1) * 100, hi * 100:(hi + 1) * 100]
    for hi in range(2):
        wb16[:c.HCW, c.W2 + hi] = w2v[hi * 100:(hi + 1) * 100]
    wb16[:, c.IDB:c.IDB + 128] = np.eye(128, dtype=np.float32)
    wb16b = np.ascontiguousarray(wb16.astype(BFNP))

    wb32 = np.zeros((128, 6), np.float32)
    wb32[:c.HCW, 0] = f32(b1)[:c.HCW]
    wb32[:c.HCW, 1] = f32(b1)[c.HCW:]
    wb32[:c.HCW, 2] = f32(mb1)[:c.HCW]
    wb32[:c.HCW, 3] = f32(mb1)[c.HCW:]
    wb32[:c.HCW, 4] = f32(mb2)[:c.HCW]
    wb32[:c.HCW, 5] = f32(mb2)[c.HCW:]

    in_maps = []
    for core in range(c.NCORES):
        b0 = core * c.BC
        # per-batch sorted token lists + per-window participant lists
        lo_tok, hi_tok = [], []          # [BC][256]
        part = {}                        # (b, m, w) -> 64 padded idx
        for b in range(c.BC):
            w = words[b0 + b]
            order = np.argsort(w, kind="stable")
            ws = w[order]
            if ws[c.HALF - 1] >= c.LO_MAX or ws[c.HALF] < c.HB:
                raise RuntimeError(
                    f"batch {b0 + b}: vocab split infeasible "
                    f"(lo_max={ws[c.HALF - 1]}, hi_min={ws[c.HALF]})")
            lo_tok.append(ws[:c.HALF])
            hi_tok.append(ws[c.HALF:] - c.HB)
            for m, pos in ((0, subj_pos), (1, obj_pos)):
                pm = (np.asarray(pos[b0 + b])[order] == 0)   # participates
                part[(b, m, 0)] = pad_slots(ws[:c.HALF][pm[:c.HALF]], c.PCAP)
                part[(b, m, 1)] = pad_slots(
                    ws[c.HALF:][pm[c.HALF:]] - c.HB, c.PCAP)
        # build per-split gather index lists
        lo_list, hi_list = [], []
        for k in range(c.nsp):
            for b in (2 * k, 2 * k + 1):
                lo_list.append(lo_tok[b])
            for m in range(2):
                lo_list.append(part[(2 * k, m, 0)])
                lo_list.append(part[(2 * k + 1, m, 0)])
            for b in (2 * k, 2 * k + 1):
                hi_list.append(hi_tok[b])
            for m in range(2):
                hi_list.append(part[(2 * k, m, 1)])
                hi_list.append(part[(2 * k + 1, m, 1)])
        nrows = c.NPC * c.nsp
        idx = np.concatenate([wrap16(np.concatenate(lo_list), nrows),
                              wrap16(np.concatenate(hi_list), nrows)], axis=1)
        in_maps.append({
            "idx": np.ascontiguousarray(idx),
            "wb16": wb16b,
            "wb32": wb32,
            "table": table,
        })
    return in_maps


def assemble_output(cfg: Cfg, results):
    c = cfg
    outs = []
    for core in range(c.NCORES):
        o = results[core]["out"]                      # [nh, HCW, BC]
        outs.append(np.transpose(o, (2, 0, 1)).reshape(c.BC, c.H))
    return np.ascontiguousarray(np.concatenate(outs, axis=0))


_CACHE = {}


def run(inputs, trace=False, **kw):
    from concourse.bass_utils import run_bass_kernel_spmd

    cfg = Cfg()
    in_maps = host_prepare(cfg, **{k: inputs[k] for k in (
        "words", "subj_pos", "obj_pos", "emb_table", "w1", "b1", "w2", "b2",
        "mw1", "mb1", "mw2", "mb2")})
    if "nc" not in _CACHE:
        _CACHE["nc"] = build_nc(cfg)
    nc = _CACHE["nc"]
    res = run_bass_kernel_spmd(nc, in_maps, core_ids=list(range(cfg.NCORES)),
                               trace=trace, **kw)
    return assemble_output(cfg, res.results), res


def kernel(**inputs) -> np.ndarray:
    return run(inputs)[0]


# revision 14
# speedup vs baseline: 1.8058x; 1.0265x over previous
"""Trainium2 Bass kernel for nn_AttentionNet (pooling / ridge regime).

Model (per batch b of B=128, L=512, D=300, H=200, V=50000):
  word_emb = emb_table[words]                          [B,L,D]
  subj_emb = max over l with subj_pos[b,l]==0 of word_emb (else -1e12)
  obj_emb  = same with obj_pos
  hid  = tanh(word_emb @ w1[:D] + subj_emb @ w1[D:] + b1)
  attn = softmax(hid @ w2, axis=l)    (b2 dropped: softmax shift-invariant)
  subj_attn = sum_l attn * word_emb   (obj_attn identical -- source bug)
  out = relu(relu(cat([subj_attn, subj_attn, subj_emb, obj_emb]) @ mw1 + mb1) @ mw2 + mb2)

Sharding: pure data parallel, 16 batches per core on 8 cores; embedding
table (bf16) and the small weights replicated.

Device plan per core (16 batches, bf16 everywhere, fp32 PSUM/biases):
  - table rows padded to 384 bf16 cols (768 B = the 256B gather
    granularity); pad values 0.0 meet zero weight rows downstream.
  - the masked max-pools are precomputed as GATHERED PARTICIPANT
    SUBTILES: the host collects, per (batch, mask, vocab-window), the
    <=64 tokens that participate in the pool (pos==0) and pads them to
    64 slots by repeating the first participant (max unchanged).  A
    batch PAIR's two 64-slot sets form one 128-row subtile.  The pool is
    then 3 PE transposes + a half-range PSUM max-reduce per subtile --
    no mask tensors, no mask arithmetic anywhere.
  - scores exp is fused into the PSUM->SBUF score copy (softmax without
    max-shift: |score| <= ~10 so exp is safe in bf16/f32), normalization
    happens batch-major after one small DMA.
  - gathers are issued first (shared num_idxs register, identity shipped
    inside the weight blob so gpsimd never switches ucode libraries for
    compute) and the per-group compute chases gather pairs.
  - per group of 4 batches: PE-transpose embeddings to D-major, dense
    hid/score matmuls, per-group softmax + attention-weighted sum so the
    attention tail pipelines with the next group's matmul stream.
"""

import numpy as np
import ml_dtypes

import concourse.bass as bass
import concourse.bacc as bacc
import concourse.mybir as mybir
import concourse.tile as tile
from contextlib import ExitStack

F32 = mybir.dt.float32
BF16 = mybir.dt.bfloat16
I16 = mybir.dt.int16

NEG_INF = 1e12      # reference constant
BFNP = ml_dtypes.bfloat16

AX = mybir.AxisListType.X
MAX = mybir.AluOpType.max
ADD = mybir.AluOpType.add
AF = mybir.ActivationFunctionType


class Cfg:
    def __init__(self):
        self.B, self.L, self.D, self.H, self.V = 128, 512, 300, 200, 50000
        self.NCORES = 8
        self.BC = 16                  # batches per core
        self.NSUB = 4                 # token subtiles per batch
        self.DP = 384                 # padded D (3 chunks of 128)
        self.E = 384                  # gather row elems (bf16) = 768 B
        self.HCW = 100                # H chunk width
        self.nh = 2
        self.nd = 3
        self.GB = 4                   # batches per group
        self.NG = 4
        self.HALF = 256               # tokens per vocab half
        self.LO_MAX = 32768
        self.HB = self.V - 32768      # high window base
        self.nsp = 8                  # gather splits per half
        self.PCAP = 64                # participant slots per (batch, window)
        # per half-split: 4 batch subtiles + 2 participant subtiles
        self.SPS = 6
        self.NS = 2 * self.nsp * self.SPS     # 96 subtiles
        self.NPC = 128 * self.SPS             # 1536 rows per gather
        self.dch = [(0, 128), (128, 128), (256, 128)]
        # wb16 blob column offsets
        self.W1A = 0
        self.W1B = 600
        self.MW1 = 1200
        self.MW2 = 3000
        self.W2 = 3400
        self.IDB = 3402
        self.NC16 = self.IDB + 128

    def sb_lo(self, b):
        return 6 * (b // 2) + 2 * (b % 2)

    def subtiles(self, b):
        lo = self.sb_lo(b)
        return [lo, lo + 1, 48 + lo, 48 + lo + 1]

    def pp(self, p, m, w):
        """Participant subtile for batch pair p, mask m, window w."""
        return (0 if w == 0 else 48) + 6 * p + 4 + m


def build_nc(cfg: Cfg):
    c = cfg
    nc = bacc.Bacc(num_swdge_queues=4)

    nix = c.NPC * c.nsp // 16                 # idx cols per half
    idx_d = nc.declare_dram_parameter("idx", [128, 2 * nix], I16, isOutput=False)
    wb16_d = nc.declare_dram_parameter("wb16", [128, c.NC16], BF16, isOutput=False)
    wb32_d = nc.declare_dram_parameter("wb32", [128, 6], F32, isOutput=False)
    table = nc.declare_dram_parameter("table", [c.V, c.E], BF16, isOutput=False)
    out_d = nc.declare_dram_parameter("out", [c.nh, c.HCW, c.BC], F32, isOutput=True)

    with tile.TileContext(nc) as tc, ExitStack() as ctx:
        sb = ctx.enter_context(tc.tile_pool(name="sb", bufs=1))

        ix_sb = sb.tile([128, 2 * nix], I16)
        wb16_sb = sb.tile([128, c.NC16], BF16)
        wb32_sb = sb.tile([128, 6], F32)
        emb_tok = sb.tile([128, c.NS, c.E], BF16)
        pooledf = sb.tile([128, 2, c.nd, c.BC], BF16)
        pooled2 = sb.tile([128, 2, c.nd, c.BC], BF16)
        pooled = sb.tile([128, 2, c.nd, c.BC], BF16)
        bias_sb = sb.tile([c.HCW, c.nh, c.BC], F32)
        attn_t = sb.tile([128, c.NSUB, c.BC], BF16)
        sattn_sb = sb.tile([c.BC, c.E], BF16)
        satd = sb.tile([128, c.nd, c.BC], BF16)
        o1_sb = sb.tile([c.HCW, c.nh, c.BC], BF16)
        out_sb = sb.tile([c.HCW, c.nh, c.BC], F32)

        identb = wb16_sb[:, c.IDB:c.IDB + 128]

        # index DMA first: it gates gather descriptor generation
        nc.sync.dma_start(out=ix_sb[:], in_=idx_d[:])
        nc.scalar.dma_start(out=wb16_sb[:], in_=wb16_d[:])
        nc.scalar.dma_start(out=wb32_sb[:], in_=wb32_d[:])

        # gathers: lo/hi pair per split k; pair k feeds batch group k
        npc = c.NPC
        nreg = nc.gpsimd.to_reg(npc)
        for k in range(c.nsp):
            i0 = k * (npc // 16)
            nc.gpsimd.dma_gather(
                out_ap=emb_tok[:, k * c.SPS:(k + 1) * c.SPS, :],
                in_ap=table[0:c.LO_MAX, :],
                idxs_ap=ix_sb[:, i0:i0 + npc // 16],
                num_idxs=npc, num_idxs_reg=nreg, elem_size=c.E,
                single_packet=True, queue_num=(2 * k) % 4)
            nc.gpsimd.dma_gather(
                out_ap=emb_tok[:, 48 + k * c.SPS:48 + (k + 1) * c.SPS, :],
                in_ap=table[c.HB:c.V, :],
                idxs_ap=ix_sb[:, nix + i0:nix + i0 + npc // 16],
                num_idxs=npc, num_idxs_reg=nreg, elem_size=c.E,
                single_packet=True, queue_num=(2 * k + 1) % 4)

        def w16(col, n, p=128):
            return wb16_sb[0:p, col:col + n]

        with tc.tile_pool(name="edpool", bufs=8) as edpool, \
             tc.tile_pool(name="hidpool", bufs=2) as hidpool, \
             tc.tile_pool(name="sstpool", bufs=2) as sstpool, \
             tc.tile_pool(name="saspool", bufs=2) as saspool, \
             tc.tile_pool(name="scgpool", bufs=2) as scgpool, \
             tc.tile_pool(name="smpool", bufs=2) as smpool, \
             tc.tile_pool(name="tepool", bufs=2, space="PSUM") as tepool, \
             tc.tile_pool(name="tppool", bufs=2, space="PSUM") as tppool, \
             tc.tile_pool(name="hpool", bufs=2, space="PSUM") as hpool, \
             tc.tile_pool(name="spool", bufs=1, space="PSUM") as spool, \
             tc.tile_pool(name="wpool", bufs=1, space="PSUM") as wpool:
            ssts = {}

            def attn_phase(ag):
                a0 = ag * c.GB
                asl = slice(a0, a0 + c.GB)
                scg = scgpool.tile([c.GB, c.L], BF16, tag="sc")
                nc.sync.dma_start(out=scg[:], in_=ssts[ag][:])
                sm = smpool.tile([c.GB, 2], F32, tag="sm")
                nc.vector.tensor_reduce(out=sm[:, 0:1], in_=scg[:], axis=AX,
                                        op=ADD)
                nc.vector.reciprocal(out=sm[:, 1:2], in_=sm[:, 0:1])
                atb = scgpool.tile([c.GB, c.L], BF16, tag="atb")
                nc.any.tensor_scalar_mul(out=atb[:], in0=scg[:],
                                         scalar1=sm[:, 1:2])
                for si in range(c.NSUB):
                    tp = tppool.tile([128, c.nd, 128], BF16, tag="tp")
                    nc.tensor.transpose(
                        out=tp[:, 0, 0:c.GB],
                        in_=atb[:, si * 128:(si + 1) * 128],
                        identity=wb16_sb[0:c.GB, c.IDB:c.IDB + c.GB])
                    nc.any.tensor_copy(out=attn_t[:, si, asl],
                                       in_=tp[:, 0, 0:c.GB])
                sas = saspool.tile([1, c.GB, c.E], BF16, tag="sas")
                for bi, b in enumerate(range(a0, a0 + c.GB)):
                    pw = wpool.tile([1, c.E], F32, tag="pw")
                    for si, s in enumerate(c.subtiles(b)):
                        nc.tensor.matmul(
                            out=pw[:],
                            lhsT=attn_t[:, si, b:b + 1],
                            rhs=emb_tok[:, s, :],
                            start=(si == 0), stop=(si == c.NSUB - 1))
                    if bi % 2 == 0:
                        nc.vector.tensor_copy(out=sas[0:1, bi, :], in_=pw[:])
                    else:
                        nc.scalar.copy(out=sas[0:1, bi, :], in_=pw[:])
                nc.scalar.dma_start(out=sattn_sb[asl, :], in_=sas[:])

            for g in range(c.NG):
                g0 = g * c.GB
                gsl = slice(g0, g0 + c.GB)
                eds = []
                for b in range(g0, g0 + c.GB):
                    subs = c.subtiles(b)
                    ed = edpool.tile([128, c.nd, c.L], BF16, tag="ed")
                    for ci, (d0, dn) in enumerate(c.dch):
                        te = tepool.tile([128, c.NSUB, 128], BF16, tag="te")
                        for si, s in enumerate(subs):
                            nc.tensor.transpose(
                                out=te[:, si, :],
                                in_=emb_tok[:, s, d0:d0 + 128],
                                identity=identb)
                        if ci == 0:
                            nc.scalar.copy(out=ed[:, ci, :], in_=te[:])
                        else:
                            nc.vector.tensor_copy(out=ed[:, ci, :], in_=te[:])
                    eds.append(ed)
                # pools from participant subtiles (batch pairs 2g, 2g+1)
                for pi in range(2):
                    p = 2 * g + pi
                    for m in range(2):
                        for w, dst in ((0, pooledf), (1, pooled2)):
                            tp = tppool.tile([128, c.nd, 128], BF16, tag="tp")
                            s = c.pp(p, m, w)
                            for ci, (d0, dn) in enumerate(c.dch):
                                nc.tensor.transpose(
                                    out=tp[:, ci, :],
                                    in_=emb_tok[:, s, d0:d0 + 128],
                                    identity=identb)
                            nc.vector.tensor_reduce(
                                out=dst[:, m, :, 2 * p], in_=tp[:, :, 0:c.PCAP],
                                axis=AX, op=MAX)
                            nc.vector.tensor_reduce(
                                out=dst[:, m, :, 2 * p + 1],
                                in_=tp[:, :, c.PCAP:128],
                                axis=AX, op=MAX)
                # combine windows (clamp unnecessary: participants are real)
                nc.any.tensor_tensor(
                    out=pooled[:, :, :, gsl], in0=pooledf[:, :, :, gsl],
                    in1=pooled2[:, :, :, gsl], op=MAX)
                # tanh bias for the group: w1b^T subj_emb + b1
                for hi in range(c.nh):
                    pb = hpool.tile([c.HCW, c.L], F32, tag="ph")
                    for ci in range(c.nd):
                        nc.tensor.matmul(
                            out=pb[:, 0:c.GB],
                            lhsT=w16(c.W1B + ci * 200 + hi * 100, 100),
                            rhs=pooled[:, 0, ci, gsl],
                            start=(ci == 0), stop=(ci == c.nd - 1))
                    nc.scalar.activation(
                        out=bias_sb[:, hi, gsl], in_=pb[:, 0:c.GB],
                        func=AF.Identity, bias=wb32_sb[0:c.HCW, hi:hi + 1],
                        scale=1.0)
                if g > 0:
                    attn_phase(g - 1)
                # dense hid + scores per batch; exp fused into score copy
                sst = sstpool.tile([1, c.GB, c.L], BF16, tag="sst")
                ssts[g] = sst
                for bi, b in enumerate(range(g0, g0 + c.GB)):
                    ed = eds[bi]
                    hid = hidpool.tile([c.HCW, c.nh, c.L], BF16, tag="hid")
                    for hi in range(c.nh):
                        ph = hpool.tile([c.HCW, c.L], F32, tag="ph")
                        for ci in range(c.nd):
                            nc.tensor.matmul(
                                out=ph[:],
                                lhsT=w16(c.W1A + ci * 200 + hi * 100, 100),
                                rhs=ed[:, ci, :],
                                start=(ci == 0), stop=(ci == c.nd - 1))
                        nc.scalar.activation(
                            out=hid[:, hi, :], in_=ph[:],
                            func=AF.Tanh, bias=bias_sb[:, hi, b:b + 1],
                            scale=1.0)
                    ps = spool.tile([1, c.L], F32, tag="ps")
                    for hi in range(c.nh):
                        nc.tensor.matmul(
                            out=ps[:],
                            lhsT=wb16_sb[0:c.HCW, c.W2 + hi:c.W2 + hi + 1],
                            rhs=hid[:, hi, :],
                            start=(hi == 0), stop=(hi == c.nh - 1))
                    nc.scalar.activation(out=sst[0:1, bi, :], in_=ps[:],
                                         func=AF.Exp)

            attn_phase(c.NG - 1)

        # ---- tail: satd transposes + output MLP
        with tc.tile_pool(name="stpool", bufs=1, space="PSUM") as stpool, \
             tc.tile_pool(name="mpool", bufs=2, space="PSUM") as mpool:
            for ci, (d0, dn) in enumerate(c.dch):
                pst = stpool.tile([128, c.BC], BF16, tag="pst")
                nc.tensor.transpose(
                    out=pst[:],
                    in_=sattn_sb[:, d0:d0 + 128],
                    identity=wb16_sb[0:c.BC, c.IDB:c.IDB + c.BC])
                nc.any.tensor_copy(out=satd[:, ci, :], in_=pst[:])
            for hi in range(c.nh):
                pm = mpool.tile([c.HCW, c.BC], F32, tag="pm")
                k = 0
                for blk in range(3):
                    for ci in range(c.nd):
                        rhs = satd[:, ci, :] if blk == 0 else pooled[:, blk - 1, ci, :]
                        nc.tensor.matmul(
                            out=pm[:],
                            lhsT=w16(c.MW1 + (blk * 3 + ci) * 200 + hi * 100, 100),
                            rhs=rhs, start=(k == 0), stop=(k == 8))
                        k += 1
                nc.scalar.activation(
                    out=o1_sb[:, hi, :], in_=pm[:], func=AF.Relu,
                    bias=wb32_sb[0:c.HCW, 2 + hi:3 + hi], scale=1.0)
            for hi in range(c.nh):
                pm2 = mpool.tile([c.HCW, c.BC], F32, tag="pm2")
                for ki in range(c.nh):
                    nc.tensor.matmul(
                        out=pm2[:],
                        lhsT=wb16_sb[0:c.HCW,
                                     c.MW2 + (ki * 2 + hi) * 100:
                                     c.MW2 + (ki * 2 + hi) * 100 + 100],
                        rhs=o1_sb[:, ki, :],
                        start=(ki == 0), stop=(ki == c.nh - 1))
                nc.scalar.activation(
                    out=out_sb[:, hi, :], in_=pm2[:], func=AF.Relu,
                    bias=wb32_sb[0:c.HCW, 4 + hi:5 + hi], scale=1.0)
            for hi in range(c.nh):
                nc.sync.dma_start(out=out_d[hi], in_=out_sb[:, hi, :])

    nc.finalize()
    return nc


# ------------------------------------------------------------------ host


def wrap16(idx, n):
    """int16 index list -> [128, n/16] wrapped + replicated per Q7 core."""
    return np.ascontiguousarray(
        np.tile(idx.astype(np.int16).reshape(n // 16, 16).T, (8, 1)))


def pad_slots(vals, cap):
    """Pad participant index list to cap slots by repeating the first."""
    assert 1 <= len(vals) <= cap, len(vals)
    out = np.empty(cap, np.int64)
    out[:len(vals)] = vals
    out[len(vals):] = vals[0]
    return out


def host_prepare(cfg: Cfg, words, subj_pos, obj_pos, emb_table,
                 w1, b1, w2, b2, mw1, mb1, mw2, mb2):
    c = cfg
    words = np.asarray(words).astype(np.int64)
    subj_pos = np.asarray(subj_pos)
    obj_pos = np.asarray(obj_pos)
    f32 = lambda x: np.asarray(x, dtype=np.float32)

    table = np.zeros((c.V, c.E), BFNP)
    table[:, :c.D] = f32(emb_table).astype(BFNP)

    w1 = f32(w1)
    mw1 = f32(mw1)
    mw1e = np.concatenate([mw1[0:c.D] + mw1[c.D:2 * c.D],
                           mw1[2 * c.D:3 * c.D], mw1[3 * c.D:4 * c.D]], axis=0)
    mw2 = f32(mw2)
    w2v = f32(w2).reshape(c.H)

    def padD(m):  # [300, H] -> [384, H]
        out = np.zeros((c.DP, m.shape[1]), np.float32)
        out[:c.D] = m
        return out

    w1a = padD(w1[:c.D])
    w1b = padD(w1[c.D:2 * c.D])
    mw1p = np.concatenate([padD(mw1e[i * c.D:(i + 1) * c.D]) for i in range(3)],
                          axis=0)  # [1152, 200]

    wb16 = np.zeros((128, c.NC16), np.float32)
    for ci in range(3):
        wb16[:, c.W1A + ci * 200:c.W1A + (ci + 1) * 200] = w1a[ci * 128:(ci + 1) * 128]
        wb16[:, c.W1B + ci * 200:c.W1B + (ci + 1) * 200] = w1b[ci * 128:(ci + 1) * 128]
    for kk in range(9):
        wb16[:, c.MW1 + kk * 200:c.MW1 + (kk + 1) * 200] = \
            mw1p[kk * 128:(kk + 1) * 128]
    for ki in range(2):
        for hi in range(2):
            wb16[:c.HCW, c.MW2 + (ki * 2 + hi) * 100:
                 c.MW2 + (ki * 2 + hi) * 100 + 100] = \
                mw2[ki * 100:(ki + 1) * 100, hi * 100:(hi + 1) * 100]
    for hi in range(2):
        wb16[:c.HCW, c.W2 + hi] = w2v[hi * 100:(hi + 1) * 100]
    wb16[:, c.IDB:c.IDB + 128] = np.eye(128, dtype=np.float32)
    wb16b = np.ascontiguousarray(wb16.astype(BFNP))

    wb32 = np.zeros((128, 6), np.float32)
    wb32[:c.HCW, 0] = f32(b1)[:c.HCW]
    wb32[:c.HCW, 1] = f32(b1)[c.HCW:]
    wb32[:c.HCW, 2] = f32(mb1)[:c.HCW]
    wb32[:c.HCW, 3] = f32(mb1)[c.HCW:]
    wb32[:c.HCW, 4] = f32(mb2)[:c.HCW]
    wb32[:c.HCW, 5] = f32(mb2)[c.HCW:]

    in_maps = []
    for core in range(c.NCORES):
        b0 = core * c.BC
        # per-batch sorted token lists + per-window participant lists
        lo_tok, hi_tok = [], []          # [BC][256]
        part = {}                        # (b, m, w) -> 64 padded idx
        for b in range(c.BC):
            w = words[b0 + b]
            order = np.argsort(w, kind="stable")
            ws = w[order]
            if ws[c.HALF - 1] >= c.LO_MAX or ws[c.HALF] < c.HB:
                raise RuntimeError(
                    f"batch {b0 + b}: vocab split infeasible "
                    f"(lo_max={ws[c.HALF - 1]}, hi_min={ws[c.HALF]})")
            lo_tok.append(ws[:c.HALF])
            hi_tok.append(ws[c.HALF:] - c.HB)
            for m, pos in ((0, subj_pos), (1, obj_pos)):
                pm = (np.asarray(pos[b0 + b])[order] == 0)   # participates
                part[(b, m, 0)] = pad_slots(ws[:c.HALF][pm[:c.HALF]], c.PCAP)
                part[(b, m, 1)] = pad_slots(
                    ws[c.HALF:][pm[c.HALF:]] - c.HB, c.PCAP)
        # build per-split gather index lists
        lo_list, hi_list = [], []
        for k in range(c.nsp):
            for b in (2 * k, 2 * k + 1):
                lo_list.append(lo_tok[b])
            for m in range(2):
                lo_list.append(part[(2 * k, m, 0)])
                lo_list.append(part[(2 * k + 1, m, 0)])
            for b in (2 * k, 2 * k + 1):
                hi_list.append(hi_tok[b])
            for m in range(2):
                hi_list.append(part[(2 * k, m, 1)])
                hi_list.append(part[(2 * k + 1, m, 1)])
        nrows = c.NPC * c.nsp
        idx = np.concatenate([wrap16(np.concatenate(lo_list), nrows),
                              wrap16(np.concatenate(hi_list), nrows)], axis=1)
        in_maps.append({
            "idx": np.ascontiguousarray(idx),
            "wb16": wb16b,
            "wb32": wb32,
            "table": table,
        })
    return in_maps


def assemble_output(cfg: Cfg, results):
    c = cfg
    outs = []
    for core in range(c.NCORES):
        o = results[core]["out"]                      # [nh, HCW, BC]
        outs.append(np.transpose(o, (2, 0, 1)).reshape(c.BC, c.H))
    return np.ascontiguousarray(np.concatenate(outs, axis=0))


_CACHE = {}


def run(inputs, trace=False, **kw):
    from concourse.bass_utils import run_bass_kernel_spmd

    cfg = Cfg()
    in_maps = host_prepare(cfg, **{k: inputs[k] for k in (
        "words", "subj_pos", "obj_pos", "emb_table", "w1", "b1", "w2", "b2",
        "mw1", "mb1", "mw2", "mb2")})
    if "nc" not in _CACHE:
        _CACHE["nc"] = build_nc(cfg)
    nc = _CACHE["nc"]
    res = run_bass_kernel_spmd(nc, in_maps, core_ids=list(range(cfg.NCORES)),
                               trace=trace, **kw)
    return assemble_output(cfg, res.results), res


def kernel(**inputs) -> np.ndarray:
    return run(inputs)[0]
